# revision 1
# baseline (speedup 1.0000x reference)
"""Trainium2 Bass kernel for nn_GATSuper (3-layer GAT + encoder/decoder MLPs).

Strategy (8 NeuronCores, SPMD):
  - Nodes sharded: core c owns global nodes [c*6250, (c+1)*6250), padded to 6272.
  - Edges (incl. self loops) partitioned by dst owner; within a core, grouped
    by dst-block (128 dst nodes) and by src table half (node table split in
    two halves of 25088 rows so dma_gather's int16 indices stay positive).
  - Per layer: node-parallel W-matmul produces h' | al_s | al_d; hcat
    ([h'|al_s] as f16 rows of 384) is AllGathered; edge phase gathers
    hcat[src] rows per edge slot (dma_gather), computes
    w = exp(leaky_relu(al_s[src]+al_d[dst])), weights features by w, and
    aggregates per dst-block with a one-hot (dstloc==iota) matmul on the PE
    which also produces the softmax denominators. Softmax division, bias and
    ELU applied per block.
  - Global mean pool partial sums per core via matmul with a one-hot batch
    matrix; host sums partials, divides by counts and runs the decoder MLP.
"""
import sys

import ml_dtypes
import numpy as np

sys.path.insert(0, "/opt/trn_rl_repo")

from concourse import bass, bacc, mybir, tile  # noqa: E402
from concourse.bass_utils import run_bass_kernel_spmd  # noqa: E402

# ---------------- problem constants (hardcoded) ----------------
N, E, IN, HID, H, OUT, G = 50000, 800000, 128, 64, 4, 40, 8
D = HID * H  # 256
NEG_SLOPE = 0.2
EPS = 1e-5
NC = 8          # cores
P = 128
NPC = N // NC   # 6250 real nodes per core
NB = 49         # dst blocks per core (ceil(6250/128))
NPAD = NB * P   # 6272 padded nodes per core
VTOT = NC * NPAD       # 50176 rows in gathered node table
THALF = VTOT // 2      # 25088 rows per half table
ROWF = 384             # f16 elements per hcat row (768B): [h' 256 | al_s 4 | pad]
ADROW = 128            # f16 elements per al_d table row (256B)
PAD_DLOC = 999.0

F32 = mybir.dt.float32
F16 = mybir.dt.float16
F8 = mybir.dt.float8e4
I16 = mybir.dt.int16
AFT = mybir.ActivationFunctionType
ALU = mybir.AluOpType

TRACE = False
STOP = 99
LAST_RESULTS = {}

_CACHE = {}


# ================= host-side schedule =================

def _build_schedule(edge_index):
    """Partition edges; build per-core gather index / dstloc arrays.

    Returns dict with per-core arrays and the static (shared) tile schedule.
    """
    src = np.concatenate([edge_index[0], np.arange(N, dtype=np.int64)])
    dst = np.concatenate([edge_index[1], np.arange(N, dtype=np.int64)])

    owner = dst // NPC
    blk = (dst % NPC) // P
    dloc = (dst % NPC) % P
    gid_src = (src // NPC) * NPAD + (src % NPC)
    half = (gid_src >= THALF).astype(np.int64)
    tabidx = gid_src - half * THALF          # < 32768, int16-safe
    dst_local = blk * P + dloc               # index into core-local al_d table

    # key = ((owner*NB + blk)*2 + half) ; count per key
    key = ((owner * NB + blk) * 2 + half)
    nkeys = NC * NB * 2
    counts = np.bincount(key, minlength=nkeys).reshape(NC, NB, 2)

    # uniform tiles per (block, half) across cores
    T = np.ceil(counts.max(axis=0) / P).astype(np.int64)  # [NB, 2]
    T = np.maximum(T, 1)

    # group blocks in pairs
    groups = [tuple(b for b in (2 * g, 2 * g + 1) if b < NB)
              for g in range((NB + 1) // 2)]

    # global chunk order: per group: A(b0),A(b1),B(b0),B(b1)
    chunk_ranges = {}   # (b, half) -> (chunk_start, n_tiles) in global order
    acc = 0
    for grp in groups:
        for h in (0, 1):
            for b in grp:
                chunk_ranges[(b, h)] = (acc, int(T[b, h]))
                acc += int(T[b, h])
    Ttot = acc
    slots_tot = Ttot * P

    # per-core slot arrays
    order = np.lexsort((half, blk, owner))  # sort by owner, blk, half
    src_sorted = tabidx[order]
    dl_sorted = dloc[order]
    dstl_sorted = dst_local[order]
    own_sorted = owner[order]
    blk_sorted = blk[order]
    half_sorted = half[order]

    # start offset of each (core, blk, half) run in sorted arrays
    k_sorted = ((own_sorted * NB + blk_sorted) * 2 + half_sorted)
    run_starts = np.searchsorted(k_sorted, np.arange(nkeys))
    run_ends = np.searchsorted(k_sorted, np.arange(nkeys) + 1)

    per_core = []
    for c in range(NC):
        slot_src = np.zeros(slots_tot, np.int16)      # table index per slot
        slot_dl = np.full(slots_tot, PAD_DLOC, np.float32)
        slot_dst = np.zeros(slots_tot, np.int16)      # al_d row per slot
        slot_half = np.zeros(slots_tot, np.int8)
        for b in range(NB):
            for h in (0, 1):
                kidx = (c * NB + b) * 2 + h
                s, e = run_starts[kidx], run_ends[kidx]
                n = e - s
                c0, nt = chunk_ranges[(b, h)]
                off = c0 * P
                slot_src[off:off + n] = src_sorted[s:e]
                slot_dl[off:off + n] = dl_sorted[s:e]
                slot_dst[off:off + n] = dstl_sorted[s:e]
                slot_half[off:off + nt * P] = h
        # wrapped int16 index arrays per half; each gather call covers a
        # contiguous col range; wrap is per call but calls cover whole-
        # chunk ranges so a single global (s*16+p) wrap works as long as
        # each call starts at a multiple of 16 slots (always: tiles of 128).
        def wrap(a):
            return a.reshape(-1, 16).T.copy()  # [16, n/16]

        maskA = slot_half == 0
        idxA = wrap(slot_src[maskA])
        idxB = wrap(slot_src[~maskA])
        idxD = wrap(slot_dst)
        # replicate to 128 partitions
        idxA = np.tile(idxA, (8, 1))
        idxB = np.tile(idxB, (8, 1))
        idxD = np.tile(idxD, (8, 1))
        dstloc = slot_dl.reshape(Ttot, P).T.copy()    # [128, Ttot]
        # one-hot S0 tiles: s0[p, t*128+d] = (dstloc[p,t]==d)  (f16)
        dl_i = slot_dl.reshape(Ttot, P).astype(np.int64)      # [T, 128] slot dloc
        s0 = np.zeros((Ttot, P, P), ml_dtypes.float8_e4m3)    # [T, e, d]
        tt, ee = np.nonzero(dl_i < P)
        s0[tt, ee, dl_i[tt, ee]] = 1.0
        s0_in = s0.transpose(1, 0, 2).reshape(P, Ttot * P).copy()
        s0t_in = s0.transpose(2, 0, 1).reshape(P, Ttot * P).copy()
        per_core.append(dict(idxA=idxA, idxB=idxB, idxD=idxD, dstloc=dstloc,
                             s0=s0_in, s0t=s0t_in))

    # per-(b,h) col offsets within the A/B wrapped arrays (in slots)
    # A-array order: groups ascending, within group A(b0),A(b1)
    a_off = {}
    b_off = {}
    accA = accB = 0
    for grp in groups:
        for b in grp:
            a_off[b] = accA
            accA += int(T[b, 0]) * P
        for b in grp:
            b_off[b] = accB
            accB += int(T[b, 1]) * P
    d_off = {}
    for grp in groups:
        base = chunk_ranges[(grp[0], 0)][0] * P
        d_off[grp] = base

    return dict(T=T, groups=groups, chunk_ranges=chunk_ranges, Ttot=Ttot,
                a_off=a_off, b_off=b_off, slots_tot=slots_tot,
                nA=accA, nB=accB, per_core=per_core)


# ================= bass program =================

def _build_bass(sch, stop=99):
    T = sch["T"]
    groups = sch["groups"]
    chunk_ranges = sch["chunk_ranges"]
    Ttot = sch["Ttot"]
    nA, nB = sch["nA"], sch["nB"]

    nc = bacc.Bacc(None, target_bir_lowering=False, num_devices=NC,
                   num_swdge_queues=4)

    # ---- inputs
    xT = nc.dram_tensor("xT", [P, NPAD], F32, kind="ExternalInput")
    idxA = nc.dram_tensor("idxA", [P, nA // 16], I16, kind="ExternalInput")
    idxB = nc.dram_tensor("idxB", [P, nB // 16], I16, kind="ExternalInput")
    s0_in = nc.dram_tensor("s0", [P, Ttot * P], F8, kind="ExternalInput")
    s0t_in = nc.dram_tensor("s0t", [P, Ttot * P], F8, kind="ExternalInput")
    eye_in = nc.dram_tensor("eye", [P, P], F32, kind="ExternalInput")
    encw1 = nc.dram_tensor("encw1", [IN, HID], F32, kind="ExternalInput")
    encw2 = nc.dram_tensor("encw2", [HID, HID], F32, kind="ExternalInput")
    b1r_in = nc.dram_tensor("b1r", [P, HID], F32, kind="ExternalInput")
    gr_in = nc.dram_tensor("gr", [P, HID], F32, kind="ExternalInput")
    ber_in = nc.dram_tensor("ber", [P, HID], F32, kind="ExternalInput")
    b2r_in = nc.dram_tensor("b2r", [P, HID], F32, kind="ExternalInput")
    rhs_in = [nc.dram_tensor(f"rhs{l}", [HID if l == 0 else D, D + 2 * H],
                             F32, kind="ExternalInput") for l in range(3)]
    brep_in = [nc.dram_tensor(f"brep{l}", [P, D], F32, kind="ExternalInput")
               for l in range(3)]
    bpool_in = nc.dram_tensor("bpool", [NPAD, G], F32, kind="ExternalInput")

    pooled_out = nc.dram_tensor("pooled", [G, D], F32, kind="ExternalOutput")

    with tile.TileContext(nc) as tc:
        with tc.tile_pool(name="const", bufs=1) as cst, \
             tc.tile_pool(name="hwork", bufs=3) as hwork, \
             tc.tile_pool(name="lhsT", bufs=3) as lhsp, \
             tc.tile_pool(name="hcat", bufs=3) as hcatp, \
             tc.tile_pool(name="gbuf", bufs=3) as gbuf, \
             tc.tile_pool(name="adbuf", bufs=2) as adbuf, \
             tc.tile_pool(name="s0b", bufs=3) as s0buf, \
             tc.tile_pool(name="small", bufs=4) as smallp, \
             tc.tile_pool(name="outp", bufs=3) as outp, \
             tc.tile_pool(name="pt", bufs=1, space="PSUM") as pt, \
             tc.tile_pool(name="pw", bufs=1, space="PSUM") as pw, \
             tc.tile_pool(name="pe", bufs=3, space="PSUM") as pep, \
             tc.tile_pool(name="pad", bufs=2, space="PSUM") as pad, \
             tc.tile_pool(name="pp", bufs=1, space="PSUM") as ppool, \
             tc.tile_pool(name="dram", bufs=1, space="DRAM") as dram:

            # ---- load constants
            def load(t_in, shape, nm, dt=F32):
                t = cst.tile(shape, dt, name=nm)
                nc.sync.dma_start(t[:], t_in[:])
                return t

            xT_t = load(xT, [P, NPAD], "xT_t")
            idxA_t = load(idxA, [P, nA // 16], "idxA_t", I16)
            idxB_t = load(idxB, [P, nB // 16], "idxB_t", I16)
            eye_t = load(eye_in, [P, P], "eye_t")
            encw1_t = load(encw1, [IN, HID], "encw1_t")
            encw2_t = load(encw2, [HID, HID], "encw2_t")
            b1r_t = load(b1r_in, [P, HID], "b1r_t")
            gr_t = load(gr_in, [P, HID], "gr_t")
            ber_t = load(ber_in, [P, HID], "ber_t")
            b2r_t = load(b2r_in, [P, HID], "b2r_t")
            rhs_t = []
            for l in range(3):
                if l == 0:
                    r0 = cst.tile([HID, D + 2 * H], F32, name=f"rhsL{l}")
                    nc.sync.dma_start(r0[:], rhs_in[l][:])
                    rhs_t.append([r0])
                else:
                    chunks = []
                    for cch in range(D // P):
                        rc = cst.tile([P, D + 2 * H], F32,
                                      name=f"rhsL{l}c{cch}")
                        nc.sync.dma_start(
                            rc[:], rhs_in[l][cch * P:(cch + 1) * P, :])
                        chunks.append(rc)
                    rhs_t.append(chunks)
            brep_t = [load(brep_in[l], [P, D], f"brep_t{l}") for l in range(3)]
            h0_t = cst.tile([P, NB * HID], F32)  # encoder output, SBUF-resident
            ald_t = cst.tile([P, NB * H], F16)   # per-layer al_d, SBUF-resident

            # ---- DRAM scratch
            h_dram = [dram.tile([NPAD, D], F32, name="hdram0"),
                      dram.tile([NPAD, D], F32, name="hdram1")]
            hcat_own_l = [dram.tile([NPAD, ROWF], F16, name=f"hcown{l}")
                          for l in range(3)]
            hcat_full_l = [dram.tile([VTOT, ROWF], F16,
                                     name=f"hcfull{l}") for l in range(3)]

            # ================ encoder ================
            for n in range(NB):
                psum1 = pw.tile([P, HID], F32, space="PSUM", tag="pw")
                nc.tensor.matmul(psum1[:], lhsT=xT_t[:, n * P:(n + 1) * P],
                                 rhs=encw1_t[:], start=True, stop=True)
                t = hwork.tile([P, HID], F32, tag="enc")
                nc.vector.tensor_tensor(out=t[:], in0=psum1[:], in1=b1r_t[:],
                                        op=ALU.add)
                # layernorm over HID
                mean = smallp.tile([P, 1], F32, tag="m")
                nc.vector.reduce_sum(out=mean[:], in_=t[:],
                                     axis=mybir.AxisListType.X)
                nc.vector.tensor_scalar_mul(mean[:], mean[:], 1.0 / HID)
                nc.vector.tensor_scalar(out=t[:], in0=t[:], scalar1=mean[:],
                                        scalar2=None, op0=ALU.subtract)
                sq = hwork.tile([P, HID], F32, tag="sq")
                nc.scalar.square(sq[:], t[:])
                var = smallp.tile([P, 1], F32, tag="v")
                nc.vector.reduce_sum(out=var[:], in_=sq[:],
                                     axis=mybir.AxisListType.X)
                # rstd = 1/sqrt(var/HID + eps)
                nc.vector.tensor_scalar(out=var[:], in0=var[:],
                                        scalar1=1.0 / HID, scalar2=EPS,
                                        op0=ALU.mult, op1=ALU.add)
                nc.scalar.sqrt(var[:], var[:])
                nc.vector.reciprocal(var[:], var[:])
                nc.vector.tensor_scalar(out=t[:], in0=t[:], scalar1=var[:],
                                        scalar2=None, op0=ALU.mult)
                nc.vector.tensor_tensor(out=t[:], in0=t[:], in1=gr_t[:],
                                        op=ALU.mult)
                nc.vector.tensor_tensor(out=t[:], in0=t[:], in1=ber_t[:],
                                        op=ALU.add)
                nc.scalar.activation(t[:], t[:], AFT.Relu)
                # transpose [128, 64] -> [64, 128]
                pst = pt.tile([HID, P], F32, space="PSUM", tag="pt")
                nc.tensor.transpose(pst[:], t[:], eye_t[:])
                lt = lhsp.tile([HID, P], F32, tag="lt64")
                nc.scalar.activation(lt[:], pst[:], AFT.Copy)
                psum2 = pw.tile([P, HID], F32, space="PSUM", tag="pw")
                nc.tensor.matmul(psum2[:], lhsT=lt[:], rhs=encw2_t[:],
                                 start=True, stop=True)
                nc.vector.tensor_tensor(out=h0_t[:, n * HID:(n + 1) * HID],
                                        in0=psum2[:], in1=b2r_t[:], op=ALU.add)

            # ================ GAT layers ================
            full_layers = min(stop // 10, 3) if stop < 90 else 3
            sub = stop % 10 if stop < 90 else 9
            for l in range(3):
                if l > full_layers or (l == full_layers and stop < 90 and sub < 1):
                    break
                part = 9 if l < full_layers or stop >= 90 else sub
                hcat_own = hcat_own_l[l]
                hcat_full = hcat_full_l[l]
                F_in = HID if l == 0 else D
                nchunk = F_in // P if F_in >= P else 1
                # ---- W phase: h' | al_s | al_d per node tile
                for n in range(NB):
                    if l == 0:
                        htile = None  # use h0_t slices
                    else:
                        htile = hwork.tile([P, D], F32, tag="hin")
                        nc.sync.dma_start(htile[:],
                                          h_dram[l % 2][n * P:(n + 1) * P, :])
                    psw = pw.tile([P, D + 2 * H], F32, space="PSUM", tag="pw")
                    for cch in range(nchunk):
                        if l == 0:
                            tin = h0_t[:, n * HID:(n + 1) * HID]
                            pst = pt.tile([HID, P], F32, space="PSUM", tag="pt")
                            lt = lhsp.tile([HID, P], F32, tag="lt64")
                        else:
                            tin = htile[:, cch * P:(cch + 1) * P]
                            pst = pt.tile([P, P], F32, space="PSUM", tag="pt")
                            lt = lhsp.tile([P, P], F32, tag="lt128")
                        nc.tensor.transpose(pst[:], tin, eye_t[:])
                        nc.scalar.activation(lt[:], pst[:], AFT.Copy)
                        nc.tensor.matmul(
                            psw[:], lhsT=lt[:], rhs=rhs_t[l][cch][:],
                            start=(cch == 0), stop=(cch == nchunk - 1))
                    hcat_tile = hcatp.tile([P, ROWF], F16, tag="hc")
                    nc.scalar.activation(hcat_tile[:, 0:D + H],
                                         psw[:, 0:D + H], AFT.Copy)
                    nc.sync.dma_start(hcat_own[n * P:(n + 1) * P, :],
                                      hcat_tile[:])
                    nc.scalar.activation(ald_t[:, n * H:(n + 1) * H],
                                         psw[:, D + H:D + 2 * H], AFT.Copy)

                # ---- allgather
                if part < 2:
                    continue
                nc.gpsimd.collective_compute(
                    "AllGather", ALU.bypass,
                    replica_groups=[list(range(NC))],
                    ins=[hcat_own[:].opt()], outs=[hcat_full[:].opt()],
                )

                # ---- edge phase per group
                if part < 3:
                    continue
                qn = [0]

                def next_q():
                    qn[0] = (qn[0] + 1) % 4
                    return qn[0]
                psp = None
                if l == 2:
                    psp = ppool.tile([G, D], F32, space="PSUM", name="psp")
                for grp in groups:
                    tA = sum(int(T[b, 0]) for b in grp)
                    tB = sum(int(T[b, 1]) for b in grp)
                    cap = tA + tB
                    g0 = chunk_ranges[(grp[0], 0)][0]  # global chunk base
                    gt = gbuf.tile([P, cap * ROWF], F16, tag="g")
                    g3 = gt[:].rearrange("p (r c) -> p r c", r=cap)
                    # gather A half rows then B half rows
                    offA = sch["a_off"][grp[0]]
                    nc.gpsimd.dma_gather(
                        out_ap=g3[:, 0:tA, :],
                        in_ap=hcat_full[0:THALF, :],
                        idxs_ap=idxA_t[:, offA // 16:(offA + tA * P) // 16],
                        num_idxs=tA * P, num_idxs_reg=tA * P,
                        elem_size=ROWF, single_packet=False, queue_num=next_q())
                    offB = sch["b_off"][grp[0]]
                    nc.gpsimd.dma_gather(
                        out_ap=g3[:, tA:cap, :],
                        in_ap=hcat_full[THALF:VTOT, :],
                        idxs_ap=idxB_t[:, offB // 16:(offB + tB * P) // 16],
                        num_idxs=tB * P, num_idxs_reg=tB * P,
                        elem_size=ROWF, single_packet=False, queue_num=next_q())
                    # stream S0 / S0T tiles for this group
                    s0g = s0buf.tile([P, cap * P], F8, tag="s0g")
                    nc.scalar.dma_start(
                        s0g[:], s0_in[:, g0 * P:(g0 + cap) * P])
                    s0tg = s0buf.tile([P, cap * P], F8, tag="s0tg")
                    nc.sync.dma_start(
                        s0tg[:], s0t_in[:, g0 * P:(g0 + cap) * P])
                    # per-edge al_d via PE: ad[e, 4] = S0T_tile.T @ al_d_blk
                    adp = pad.tile([P, cap * H], F32, space="PSUM", tag="adp")
                    for b in grp:
                        for hf in (0, 1):
                            c0, nt = chunk_ranges[(b, hf)]
                            for t_ in range(nt):
                                tt_ = c0 - g0 + t_
                                nc.tensor.matmul(
                                    adp[:, tt_ * H:(tt_ + 1) * H],
                                    lhsT=s0tg[:, tt_ * P:(tt_ + 1) * P],
                                    rhs=ald_t[:, b * H:(b + 1) * H],
                                    start=True, stop=True)
                    # scores: s = al_s[src] + al_d[dst] (in G cols 256:260)
                    sl = g3[:, :, D:D + H]
                    adp3 = adp[:].rearrange("p (r c) -> p r c", r=cap)
                    nc.vector.tensor_tensor(out=sl, in0=sl, in1=adp3,
                                            op=ALU.add)
                    # leaky relu: max(s, 0.2*s)
                    tmp = smallp.tile([P, cap * H], F16, tag="lrl")
                    tmp3 = tmp[:].rearrange("p (r c) -> p r c", r=cap)
                    nc.vector.tensor_scalar_mul(tmp3, sl, NEG_SLOPE)
                    nc.vector.tensor_tensor(out=sl, in0=sl, in1=tmp3,
                                            op=ALU.max)
                    # w = exp(s)
                    nc.scalar.activation(sl, sl, AFT.Exp)
                    # weight features by w per head (3 heads DVE, 1 head gpsimd)
                    for hh in range(H):
                        in0 = g3[:, :, hh * HID:(hh + 1) * HID]
                        in1 = g3[:, :, D + hh:D + hh + 1].to_broadcast(
                            [P, cap, HID])
                        nc.vector.tensor_tensor(out=in0, in0=in0, in1=in1,
                                                op=ALU.mult)
                    # per-block aggregation matmul
                    if part < 4:
                        continue
                    for b in grp:
                        cA0, nTA = chunk_ranges[(b, 0)]
                        cB0, nTB = chunk_ranges[(b, 1)]
                        tiles = [cA0 - g0 + i for i in range(nTA)] + \
                                [cB0 - g0 + i for i in range(nTB)]
                        pse = pep.tile([P, D + H], F32, space="PSUM",
                                       tag="pe")
                        for i, t_ in enumerate(tiles):
                            nc.tensor.matmul(pse[:],
                                             lhsT=s0g[:, t_ * P:(t_ + 1) * P],
                                             rhs=g3[:, t_, 0:D + H],
                                             start=(i == 0),
                                             stop=(i == len(tiles) - 1))
                        # softmax divide + bias + ELU
                        den = smallp.tile([P, H], F32, tag="den")
                        nc.vector.tensor_scalar(out=den[:],
                                                in0=pse[:, D:D + H],
                                                scalar1=1e-16, scalar2=None,
                                                op0=ALU.add)
                        nc.vector.reciprocal(den[:], den[:])
                        xo = outp.tile([P, D], F32, tag="xo")
                        den_b = den[:].rearrange(
                            "p (h o) -> p h o", o=1).to_broadcast([P, H, HID])
                        nc.vector.tensor_tensor(
                            out=xo[:].rearrange("p (h c) -> p h c", h=H),
                            in0=pse[:, 0:D].rearrange(
                                "p (h c) -> p h c", h=H),
                            in1=den_b, op=ALU.mult)
                        nc.vector.tensor_tensor(out=xo[:], in0=xo[:],
                                                in1=brep_t[l][:], op=ALU.add)
                        # ELU: relu(x)-1 + exp(min(x,0))
                        emin = outp.tile([P, D], F32, tag="emin")
                        nc.vector.tensor_scalar_min(emin[:], xo[:], 0.0)
                        nc.scalar.activation(emin[:], emin[:], AFT.Exp)
                        nc.vector.tensor_scalar(out=xo[:], in0=xo[:],
                                                scalar1=0.0, scalar2=-1.0,
                                                op0=ALU.max, op1=ALU.add)
                        nc.vector.tensor_tensor(out=xo[:], in0=xo[:],
                                                in1=emin[:], op=ALU.add)
                        if l < 2:
                            nc.sync.dma_start(
                                h_dram[(l + 1) % 2][b * P:(b + 1) * P, :],
                                xo[:])
                        else:
                            # pooling partial: psum_pool += Bp.T @ h3
                            bp = smallp.tile([P, G], F32, tag="bp")
                            nc.sync.dma_start(
                                bp[:], bpool_in[b * P:(b + 1) * P, :])
                            nc.tensor.matmul(psp[:], lhsT=bp[:], rhs=xo[:],
                                             start=(b == 0), stop=(b == NB - 1))
                            if b == NB - 1:
                                po = outp.tile([G, D], F32, tag="po")
                                nc.scalar.activation(po[:], psp[:], AFT.Copy)
                                nc.sync.dma_start(pooled_out[:], po[:])
            if stop < 90:
                dummy = outp.tile([G, D], F32, name="dummy")
                nc.gpsimd.memset(dummy[:], 0.0)
                nc.sync.dma_start(pooled_out[:], dummy[:])
    return nc


# ================= host wrapper =================

def kernel(**inputs):
    x = np.asarray(inputs["x"], np.float32)
    edge_index = np.asarray(inputs["edge_index"]).astype(np.int64)
    batch = np.asarray(inputs["batch"]).astype(np.int64)

    if "sch" not in _CACHE:
        _CACHE["sch"] = _build_schedule(edge_index)
        _CACHE["nc"] = _build_bass(_CACHE["sch"], stop=STOP)
        _CACHE["nc"].compile()
    sch = _CACHE["sch"]
    nc = _CACHE["nc"]

    # ---- weight prep
    def a_tilde(a):  # [H, HID] -> [D, H] block diag
        m = np.zeros((D, H), np.float32)
        for h in range(H):
            m[h * HID:(h + 1) * HID, h] = a[h]
        return m

    rhs = []
    breps = []
    for l in range(3):
        W = np.asarray(inputs[f"conv{l}_w"], np.float32)
        a_s = np.asarray(inputs[f"conv{l}_as"], np.float32)
        a_d = np.asarray(inputs[f"conv{l}_ad"], np.float32)
        bb = np.asarray(inputs[f"conv{l}_b"], np.float32)
        rhs.append(np.concatenate(
            [W, W @ a_tilde(a_s), W @ a_tilde(a_d)], axis=1))
        breps.append(np.tile(bb[None, :], (P, 1)))

    eye = np.eye(P, dtype=np.float32)
    b1r = np.tile(np.asarray(inputs["enc_b1"], np.float32)[None, :], (P, 1))
    gr = np.tile(np.asarray(inputs["enc_g"], np.float32)[None, :], (P, 1))
    ber = np.tile(np.asarray(inputs["enc_be"], np.float32)[None, :], (P, 1))
    b2r = np.tile(np.asarray(inputs["enc_b2"], np.float32)[None, :], (P, 1))

    in_maps = []
    for c in range(NC):
        xc = np.zeros((NPAD, IN), np.float32)
        xc[:NPC] = x[c * NPC:(c + 1) * NPC]
        bp = np.zeros((NPAD, G), np.float32)
        bc = batch[c * NPC:(c + 1) * NPC]
        bp[np.arange(NPC), bc] = 1.0
        pc = sch["per_core"][c]
        in_maps.append({
            "xT": xc.T.copy(),
            "idxA": pc["idxA"], "idxB": pc["idxB"],
            "s0": pc["s0"], "s0t": pc["s0t"],
            "eye": eye,
            "encw1": np.asarray(inputs["enc_w1"], np.float32),
            "encw2": np.asarray(inputs["enc_w2"], np.float32),
            "b1r": b1r, "gr": gr, "ber": ber, "b2r": b2r,
            "rhs0": rhs[0], "rhs1": rhs[1], "rhs2": rhs[2],
            "brep0": breps[0], "brep1": breps[1], "brep2": breps[2],
            "bpool": bp,
        })

    LAST_RESULTS["in_maps"] = in_maps
    res = run_bass_kernel_spmd(nc, in_maps, core_ids=list(range(NC)),
                               trace=TRACE)
    LAST_RESULTS["res"] = res

    pooled = np.zeros((G, D), np.float32)
    for c in range(NC):
        pooled += res.results[c]["pooled"]
    cnt = np.bincount(batch, minlength=G).astype(np.float32)[:, None]
    pooled = pooled / np.maximum(cnt, 1.0)

    # decoder MLP on host (f32, matches reference ops)
    w1 = np.asarray(inputs["dec_w1"], np.float32)
    b1 = np.asarray(inputs["dec_b1"], np.float32)
    g_ = np.asarray(inputs["dec_g"], np.float32)
    be = np.asarray(inputs["dec_be"], np.float32)
    w2 = np.asarray(inputs["dec_w2"], np.float32)
    b2 = np.asarray(inputs["dec_b2"], np.float32)
    t = pooled @ w1 + b1
    m = t.mean(-1, keepdims=True)
    v = np.square(t - m).mean(-1, keepdims=True)
    t = g_ * (t - m) / np.sqrt(v + EPS) + be
    t = np.maximum(t, 0.0)
    out = t @ w2 + b2
    return out.astype(np.float32)



# revision 4
# speedup vs baseline: 1.4349x; 1.4349x over previous
"""Trainium2 Bass kernel for nn_GATSuper (3-layer GAT + encoder/decoder MLPs).

Strategy (8 NeuronCores, SPMD):
  - Nodes sharded: core c owns global nodes [c*6250, (c+1)*6250), padded to 6272.
  - Node feature table rows are 512B: [h' 256 fp8e4m3 | al_s 4 f16 | pad].
    fp8 features cut gather traffic 33% vs f16 and let the aggregation
    matmul run at fp8 rate.
  - The gathered table is built with TWO AllGathers per layer so each can
    overlap compute: table A = each core's node positions [0,3200) (rows
    owner*3200+pos, 25600 total), table B = positions [3200,6272) (rows
    owner*3072+(pos-3200), 24576 total). Both < 32768 so dma_gather's
    int16 indices work without further splitting. AG-A fires after the
    W phase of node blocks 0..24, AG-B after blocks 25..48.
  - Edges partitioned by dst owner; within a core, grouped per dst block
    (128 dst nodes), each block's slots = [A-half tiles | B-half tiles].
  - Per layer edge phase (per dst block): dma_gather rows per edge slot,
    per-edge al_d via one-hot S0T matmul, scores s=al_s+al_d, w =
    exp(leaky_relu(s)) (f16), w cast to fp8 in-row, features weighted by w
    per head (fp8 DVE), per-block one-hot S0 aggregation matmul (fp8) also
    yields softmax denominators; divide, bias, ELU in f16; result written
    to SBUF-resident h (f16). The next layer's W matmul for the block is
    emitted inline so AllGathers and W work hide inside the edge phase.
  - Global mean pool partials per core via one-hot matmul; host sums
    partials, divides by counts, runs the decoder MLP.
"""
import sys

import ml_dtypes
import numpy as np

sys.path.insert(0, "/opt/trn_rl_repo")

from concourse import bass, bacc, mybir, tile  # noqa: E402
from concourse.bass_utils import run_bass_kernel_spmd  # noqa: E402

# ---------------- problem constants (hardcoded) ----------------
N, E, IN, HID, H, OUT, G = 50000, 800000, 128, 64, 4, 40, 8
D = HID * H  # 256
NEG_SLOPE = 0.2
EPS = 1e-5
NC = 8          # cores
P = 128
NPC = N // NC   # 6250 real nodes per core
NB = 49         # dst blocks per core (ceil(6250/128))
NPAD = NB * P   # 6272 padded nodes per core
APOS = 3200     # node positions [0,APOS) -> table A
BPOS = NPAD - APOS  # 3072 positions -> table B
AROWS = NC * APOS   # 25600
BROWS = NC * BPOS   # 24576
ROWB = 512          # bytes (= fp8 elements) per table row
WCOL = 264          # fp8 col where w (fp8) is written per edge row
RHSW = WCOL + H     # 268: agg matmul rhs width

F32 = mybir.dt.float32
F16 = mybir.dt.float16
F8 = mybir.dt.float8e4
I16 = mybir.dt.int16
AFT = mybir.ActivationFunctionType
ALU = mybir.AluOpType

TRACE = False
LAST_RESULTS = {}

_CACHE = {}


# ================= host-side schedule =================

def _build_schedule(edge_index):
    """Partition edges; build per-core gather index / one-hot arrays."""
    src = np.concatenate([edge_index[0], np.arange(N, dtype=np.int64)])
    dst = np.concatenate([edge_index[1], np.arange(N, dtype=np.int64)])

    owner = dst // NPC
    blk = (dst % NPC) // P
    dloc = (dst % NPC) % P
    s_owner = src // NPC
    s_pos = src % NPC
    half = (s_pos >= APOS).astype(np.int64)
    tabidx = np.where(half == 0, s_owner * APOS + s_pos,
                      s_owner * BPOS + (s_pos - APOS))

    # key = ((owner*NB + blk)*2 + half); count per key
    key = ((owner * NB + blk) * 2 + half)
    nkeys = NC * NB * 2
    counts = np.bincount(key, minlength=nkeys).reshape(NC, NB, 2)

    # uniform tiles per (block, half) across cores
    T = np.ceil(counts.max(axis=0) / P).astype(np.int64)  # [NB, 2]
    T = np.maximum(T, 1)

    # global chunk order: per block b: A tiles then B tiles
    chunk_ranges = {}
    a_off = {}
    b_off = {}
    acc = accA = accB = 0
    for b in range(NB):
        chunk_ranges[(b, 0)] = (acc, int(T[b, 0]))
        acc += int(T[b, 0])
        chunk_ranges[(b, 1)] = (acc, int(T[b, 1]))
        acc += int(T[b, 1])
        a_off[b] = accA
        accA += int(T[b, 0]) * P
        b_off[b] = accB
        accB += int(T[b, 1]) * P
    Ttot = acc
    slots_tot = Ttot * P

    # per-core slot arrays
    order = np.lexsort((half, blk, owner))
    src_sorted = tabidx[order]
    dl_sorted = dloc[order]
    own_sorted = owner[order]
    blk_sorted = blk[order]
    half_sorted = half[order]

    k_sorted = ((own_sorted * NB + blk_sorted) * 2 + half_sorted)
    run_starts = np.searchsorted(k_sorted, np.arange(nkeys))
    run_ends = np.searchsorted(k_sorted, np.arange(nkeys) + 1)

    per_core = []
    for c in range(NC):
        slot_src = np.zeros(slots_tot, np.int16)
        slot_dl = np.full(slots_tot, P + 1, np.int64)  # pad -> no one-hot
        slot_half = np.zeros(slots_tot, np.int8)
        for b in range(NB):
            for h in (0, 1):
                kidx = (c * NB + b) * 2 + h
                s, e = run_starts[kidx], run_ends[kidx]
                n = e - s
                c0, nt = chunk_ranges[(b, h)]
                off = c0 * P
                slot_src[off:off + n] = src_sorted[s:e]
                slot_dl[off:off + n] = dl_sorted[s:e]
                slot_half[off:off + nt * P] = h

        def wrap(a):
            return a.reshape(-1, 16).T.copy()  # [16, n/16]

        maskA = slot_half == 0
        idxA = np.tile(wrap(slot_src[maskA]), (8, 1))
        idxB = np.tile(wrap(slot_src[~maskA]), (8, 1))
        # one-hot S0 tiles: s0[t, e, d] = (dloc[slot]==d) fp8
        dl_i = slot_dl.reshape(Ttot, P)
        s0 = np.zeros((Ttot, P, P), ml_dtypes.float8_e4m3)
        tt, ee = np.nonzero(dl_i < P)
        s0[tt, ee, dl_i[tt, ee]] = 1.0
        s0_in = s0.transpose(1, 0, 2).reshape(P, Ttot * P).copy()
        s0t_in = s0.transpose(2, 0, 1).reshape(P, Ttot * P).copy()
        per_core.append(dict(idxA=idxA, idxB=idxB, s0=s0_in, s0t=s0t_in))

    return dict(T=T, chunk_ranges=chunk_ranges, Ttot=Ttot,
                a_off=a_off, b_off=b_off, slots_tot=slots_tot,
                nA=accA, nB=accB, per_core=per_core)


# ================= bass program =================

def _build_bass(sch):
    T = sch["T"]
    chunk_ranges = sch["chunk_ranges"]
    Ttot = sch["Ttot"]
    nA, nB = sch["nA"], sch["nB"]

    nc = bacc.Bacc(None, target_bir_lowering=False, num_devices=NC,
                   num_swdge_queues=4)

    # ---- inputs
    xT = nc.dram_tensor("xT", [P, NPAD], F16, kind="ExternalInput")
    idxA = nc.dram_tensor("idxA", [P, nA // 16], I16, kind="ExternalInput")
    idxB = nc.dram_tensor("idxB", [P, nB // 16], I16, kind="ExternalInput")
    s0_in = nc.dram_tensor("s0", [P, Ttot * P], F8, kind="ExternalInput")
    s0t_in = nc.dram_tensor("s0t", [P, Ttot * P], F8, kind="ExternalInput")
    eye_in = nc.dram_tensor("eye", [P, P], F16, kind="ExternalInput")
    encw1 = nc.dram_tensor("encw1", [IN, HID], F16, kind="ExternalInput")
    encw2 = nc.dram_tensor("encw2", [HID, HID], F16, kind="ExternalInput")
    b1r_in = nc.dram_tensor("b1r", [P, HID], F32, kind="ExternalInput")
    gr_in = nc.dram_tensor("gr", [P, HID], F32, kind="ExternalInput")
    ber_in = nc.dram_tensor("ber", [P, HID], F32, kind="ExternalInput")
    b2r_in = nc.dram_tensor("b2r", [P, HID], F32, kind="ExternalInput")
    rhs_in = [nc.dram_tensor(f"rhs{l}", [HID if l == 0 else D, D + 2 * H],
                             F16, kind="ExternalInput") for l in range(3)]
    brep_in = [nc.dram_tensor(f"brep{l}", [P, D], F16, kind="ExternalInput")
               for l in range(3)]
    bpool_in = nc.dram_tensor("bpool", [P, NB * G], F16, kind="ExternalInput")

    pooled_out = nc.dram_tensor("pooled", [G, D], F32, kind="ExternalOutput")

    with tile.TileContext(nc) as tc:
        with tc.tile_pool(name="const", bufs=1) as cst, \
             tc.tile_pool(name="hwork", bufs=3) as hwork, \
             tc.tile_pool(name="lhsT", bufs=3) as lhsp, \
             tc.tile_pool(name="hcat", bufs=3) as hcatp, \
             tc.tile_pool(name="gbuf", bufs=4) as gbuf, \
             tc.tile_pool(name="s0b", bufs=4) as s0buf, \
             tc.tile_pool(name="small", bufs=4) as smallp, \
             tc.tile_pool(name="outp", bufs=3) as outp, \
             tc.tile_pool(name="pt", bufs=1, space="PSUM") as pt, \
             tc.tile_pool(name="pw", bufs=2, space="PSUM") as pw, \
             tc.tile_pool(name="pe", bufs=3, space="PSUM") as pep, \
             tc.tile_pool(name="pad", bufs=1, space="PSUM") as pad, \
             tc.tile_pool(name="pp", bufs=1, space="PSUM") as ppool, \
             tc.tile_pool(name="dram", bufs=1, space="DRAM") as dram:

            # ---- load constants
            def load(t_in, shape, nm, dt=F32):
                t = cst.tile(shape, dt, name=nm)
                nc.sync.dma_start(t[:], t_in[:])
                return t

            xT_t = load(xT, [P, NPAD], "xT_t", F16)
            idxA_t = load(idxA, [P, nA // 16], "idxA_t", I16)
            idxB_t = load(idxB, [P, nB // 16], "idxB_t", I16)
            eye_t = load(eye_in, [P, P], "eye_t", F16)
            encw1_t = load(encw1, [IN, HID], "encw1_t", F16)
            encw2_t = load(encw2, [HID, HID], "encw2_t", F16)
            b1r_t = load(b1r_in, [P, HID], "b1r_t")
            gr_t = load(gr_in, [P, HID], "gr_t")
            ber_t = load(ber_in, [P, HID], "ber_t")
            b2r_t = load(b2r_in, [P, HID], "b2r_t")
            bpool_t = load(bpool_in, [P, NB * G], "bpool_t", F16)
            rhs_t = []
            for l in range(3):
                if l == 0:
                    r0 = cst.tile([HID, D + 2 * H], F16, name=f"rhsL{l}")
                    nc.sync.dma_start(r0[:], rhs_in[l][:])
                    rhs_t.append([r0])
                else:
                    chunks = []
                    for cch in range(D // P):
                        rc = cst.tile([P, D + 2 * H], F16,
                                      name=f"rhsL{l}c{cch}")
                        nc.sync.dma_start(
                            rc[:], rhs_in[l][cch * P:(cch + 1) * P, :])
                        chunks.append(rc)
                    rhs_t.append(chunks)
            brep_t = [load(brep_in[l], [P, D], f"brep_t{l}", F16)
                      for l in range(3)]
            h0_t = cst.tile([P, NB * HID], F16)  # encoder out, SBUF-resident
            h_sb = cst.tile([P, NB * D], F16)    # GAT layer io, SBUF-resident
            ald_t = cst.tile([P, NB * H], F16)   # per-layer al_d

            # ---- DRAM scratch
            hcat_own_l = [dram.tile([NPAD, ROWB], F8, name=f"hcown{l}")
                          for l in range(3)]
            hcatA_l = [dram.tile([AROWS, ROWB], F8, name=f"hcA{l}",
                                 addr_space="Shared") for l in range(3)]
            hcatB_l = [dram.tile([BROWS, ROWB], F8, name=f"hcB{l}",
                                 addr_space="Shared") for l in range(3)]

            def emit_ag(l, half):
                own = hcat_own_l[l]
                if half == 0:
                    nc.gpsimd.collective_compute(
                        "AllGather", ALU.bypass,
                        replica_groups=[list(range(NC))],
                        ins=[own[0:APOS, :].opt()],
                        outs=[hcatA_l[l][:].opt()])
                else:
                    nc.gpsimd.collective_compute(
                        "AllGather", ALU.bypass,
                        replica_groups=[list(range(NC))],
                        ins=[own[APOS:NPAD, :].opt()],
                        outs=[hcatB_l[l][:].opt()])

            def emit_w_block(l, n, rhs_chunks):
                """W matmul for layer l, node block n -> hcat_own[l] rows."""
                psw = pw.tile([P, D + 2 * H], F32, space="PSUM", tag="pw")
                if l == 0:
                    tin = h0_t[:, n * HID:(n + 1) * HID]
                    pst = pt.tile([HID, P], F16, space="PSUM", tag="pt")
                    lt = lhsp.tile([HID, P], F16, tag="lt64")
                    nc.tensor.transpose(pst[:], tin, eye_t[:])
                    nc.scalar.activation(lt[:], pst[:], AFT.Copy)
                    nc.tensor.matmul(psw[:], lhsT=lt[:], rhs=rhs_chunks[0][:],
                                     start=True, stop=True)
                else:
                    for cch in range(2):
                        tin = h_sb[:, n * D + cch * P:n * D + (cch + 1) * P]
                        pst = pt.tile([P, P], F16, space="PSUM", tag="pt")
                        lt = lhsp.tile([P, P], F16, tag="lt128")
                        nc.tensor.transpose(pst[:], tin, eye_t[:])
                        nc.scalar.activation(lt[:], pst[:], AFT.Copy)
                        nc.tensor.matmul(psw[:], lhsT=lt[:],
                                         rhs=rhs_chunks[cch][:],
                                         start=(cch == 0), stop=(cch == 1))
                hct = hcatp.tile([P, ROWB], F8, tag="hc")
                nc.scalar.activation(hct[:, 0:D], psw[:, 0:D], AFT.Copy)
                hct16 = hct[:].bitcast(F16)
                nc.scalar.activation(hct16[:, D // 2:D // 2 + H],
                                     psw[:, D:D + H], AFT.Copy)
                nc.vector.tensor_scalar(
                    out=ald_t[:, n * H:(n + 1) * H],
                    in0=psw[:, D + H:D + 2 * H], scalar1=0.0, scalar2=None,
                    op0=ALU.add)
                nc.sync.dma_start(hcat_own_l[l][n * P:(n + 1) * P, :], hct[:])

            # ================ encoder + layer-0 W phase ================
            for n in range(NB):
                psum1 = pw.tile([P, HID], F32, space="PSUM", tag="pw")
                nc.tensor.matmul(psum1[:], lhsT=xT_t[:, n * P:(n + 1) * P],
                                 rhs=encw1_t[:], start=True, stop=True)
                t = hwork.tile([P, HID], F32, tag="enc")
                nc.vector.tensor_tensor(out=t[:], in0=psum1[:], in1=b1r_t[:],
                                        op=ALU.add)
                # layernorm over HID
                mean = smallp.tile([P, 1], F32, tag="m")
                nc.vector.reduce_sum(out=mean[:], in_=t[:],
                                     axis=mybir.AxisListType.X)
                nc.vector.tensor_scalar_mul(mean[:], mean[:], 1.0 / HID)
                nc.vector.tensor_scalar(out=t[:], in0=t[:], scalar1=mean[:],
                                        scalar2=None, op0=ALU.subtract)
                sq = hwork.tile([P, HID], F32, tag="sq")
                nc.scalar.square(sq[:], t[:])
                var = smallp.tile([P, 1], F32, tag="v")
                nc.vector.reduce_sum(out=var[:], in_=sq[:],
                                     axis=mybir.AxisListType.X)
                nc.vector.tensor_scalar(out=var[:], in0=var[:],
                                        scalar1=1.0 / HID, scalar2=EPS,
                                        op0=ALU.mult, op1=ALU.add)
                nc.scalar.sqrt(var[:], var[:])
                nc.vector.reciprocal(var[:], var[:])
                nc.vector.tensor_scalar(out=t[:], in0=t[:], scalar1=var[:],
                                        scalar2=None, op0=ALU.mult)
                nc.vector.tensor_tensor(out=t[:], in0=t[:], in1=gr_t[:],
                                        op=ALU.mult)
                nc.vector.tensor_tensor(out=t[:], in0=t[:], in1=ber_t[:],
                                        op=ALU.add)
                t16 = hwork.tile([P, HID], F16, tag="enc16")
                nc.scalar.activation(t16[:], t[:], AFT.Relu)
                pst = pt.tile([HID, P], F16, space="PSUM", tag="pt")
                nc.tensor.transpose(pst[:], t16[:], eye_t[:])
                lt = lhsp.tile([HID, P], F16, tag="lt64")
                nc.scalar.activation(lt[:], pst[:], AFT.Copy)
                psum2 = pw.tile([P, HID], F32, space="PSUM", tag="pw")
                nc.tensor.matmul(psum2[:], lhsT=lt[:], rhs=encw2_t[:],
                                 start=True, stop=True)
                nc.vector.tensor_tensor(out=h0_t[:, n * HID:(n + 1) * HID],
                                        in0=psum2[:], in1=b2r_t[:], op=ALU.add)
                emit_w_block(0, n, rhs_t[0])
                if n == 24:
                    emit_ag(0, 0)
            emit_ag(0, 1)

            # ================ GAT layers ================
            qn = [0]

            def next_q():
                qn[0] = (qn[0] + 1) % 4
                return qn[0]

            for l in range(3):
                psp = None
                if l == 2:
                    psp = ppool.tile([G, D], F32, space="PSUM", name="psp")
                for b in range(NB):
                    cA0, tA = chunk_ranges[(b, 0)]
                    cB0, tB = chunk_ranges[(b, 1)]
                    cap = tA + tB
                    g0 = cA0
                    gt = gbuf.tile([P, cap * ROWB], F8, tag="g")
                    g3 = gt[:].rearrange("p (r c) -> p r c", r=cap)
                    offA = sch["a_off"][b]
                    nc.gpsimd.dma_gather(
                        out_ap=g3[:, 0:tA, :],
                        in_ap=hcatA_l[l][:],
                        idxs_ap=idxA_t[:, offA // 16:(offA + tA * P) // 16],
                        num_idxs=tA * P, num_idxs_reg=tA * P,
                        elem_size=ROWB, single_packet=False,
                        queue_num=next_q())
                    offB = sch["b_off"][b]
                    nc.gpsimd.dma_gather(
                        out_ap=g3[:, tA:cap, :],
                        in_ap=hcatB_l[l][:],
                        idxs_ap=idxB_t[:, offB // 16:(offB + tB * P) // 16],
                        num_idxs=tB * P, num_idxs_reg=tB * P,
                        elem_size=ROWB, single_packet=False,
                        queue_num=next_q())
                    # one-hot tiles for this block
                    s0g = s0buf.tile([P, cap * P], F8, tag="s0g")
                    nc.scalar.dma_start(s0g[:], s0_in[:, g0 * P:(g0 + cap) * P])
                    s0tg = s0buf.tile([P, cap * P], F8, tag="s0tg")
                    nc.sync.dma_start(s0tg[:], s0t_in[:, g0 * P:(g0 + cap) * P])
                    # per-edge al_d via PE
                    adp = pad.tile([P, cap * H], F32, space="PSUM", tag="adp")
                    for t_ in range(cap):
                        nc.tensor.matmul(
                            adp[:, t_ * H:(t_ + 1) * H],
                            lhsT=s0tg[:, t_ * P:(t_ + 1) * P],
                            rhs=ald_t[:, b * H:(b + 1) * H],
                            start=True, stop=True)
                    # scores: s = al_s[src] + al_d[dst]  (f16 view of rows)
                    g16 = gt[:].bitcast(F16).rearrange(
                        "p (r c) -> p r c", r=cap)
                    sl = g16[:, :, D // 2:D // 2 + H]
                    adp3 = adp[:].rearrange("p (r c) -> p r c", r=cap)
                    nc.vector.tensor_tensor(out=sl, in0=sl, in1=adp3,
                                            op=ALU.add)
                    # leaky relu: max(s, 0.2*s)
                    tmp = smallp.tile([P, cap * H], F16, tag="lrl")
                    tmp3 = tmp[:].rearrange("p (r c) -> p r c", r=cap)
                    nc.vector.tensor_scalar_mul(tmp3, sl, NEG_SLOPE)
                    nc.vector.tensor_tensor(out=sl, in0=sl, in1=tmp3,
                                            op=ALU.max)
                    # w = exp(s), cast to fp8 into row col WCOL
                    nc.scalar.activation(sl, sl, AFT.Exp)
                    w8 = g3[:, :, WCOL:WCOL + H]
                    nc.scalar.activation(w8, sl, AFT.Copy)
                    # weight features by w per head (fp8 in-place)
                    for hh in range(H):
                        in0 = g3[:, :, hh * HID:(hh + 1) * HID]
                        in1 = g3[:, :, WCOL + hh:WCOL + hh + 1].to_broadcast(
                            [P, cap, HID])
                        nc.vector.tensor_tensor(out=in0, in0=in0, in1=in1,
                                                op=ALU.mult)
                    # per-block aggregation matmul (fp8)
                    pse = pep.tile([P, RHSW], F32, space="PSUM", tag="pe")
                    for i in range(cap):
                        nc.tensor.matmul(pse[:],
                                         lhsT=s0g[:, i * P:(i + 1) * P],
                                         rhs=g3[:, i, 0:RHSW],
                                         start=(i == 0), stop=(i == cap - 1))
                    # softmax divide + bias + ELU (f16)
                    den = smallp.tile([P, H], F32, tag="den")
                    nc.vector.tensor_scalar(out=den[:],
                                            in0=pse[:, WCOL:WCOL + H],
                                            scalar1=1e-16, scalar2=None,
                                            op0=ALU.add)
                    nc.vector.reciprocal(den[:], den[:])
                    xo = outp.tile([P, D], F16, tag="xo")
                    den_b = den[:].rearrange(
                        "p (h o) -> p h o", o=1).to_broadcast([P, H, HID])
                    nc.vector.tensor_tensor(
                        out=xo[:].rearrange("p (h c) -> p h c", h=H),
                        in0=pse[:, 0:D].rearrange("p (h c) -> p h c", h=H),
                        in1=den_b, op=ALU.mult)
                    nc.vector.tensor_tensor(out=xo[:], in0=xo[:],
                                            in1=brep_t[l][:], op=ALU.add)
                    # ELU: (max(x,0)-1) + exp(min(x,0))
                    emin = outp.tile([P, D], F16, tag="emin")
                    nc.vector.tensor_scalar_min(emin[:], xo[:], 0.0)
                    nc.scalar.activation(emin[:], emin[:], AFT.Exp)
                    nc.vector.tensor_scalar(out=xo[:], in0=xo[:],
                                            scalar1=0.0, scalar2=-1.0,
                                            op0=ALU.max, op1=ALU.add)
                    hout = h_sb[:, b * D:(b + 1) * D]
                    nc.vector.tensor_tensor(out=hout, in0=xo[:],
                                            in1=emin[:], op=ALU.add)
                    if l < 2:
                        # next layer W phase for this block, inline
                        emit_w_block(l + 1, b, rhs_t[l + 1])
                        if b == 24:
                            emit_ag(l + 1, 0)
                        elif b == NB - 1:
                            emit_ag(l + 1, 1)
                    else:
                        # pooling partial: psp += Bp.T @ h
                        nc.tensor.matmul(
                            psp[:], lhsT=bpool_t[:, b * G:(b + 1) * G],
                            rhs=hout, start=(b == 0), stop=(b == NB - 1))
                        if b == NB - 1:
                            po = outp.tile([G, D], F32, tag="po")
                            nc.scalar.activation(po[:], psp[:], AFT.Copy)
                            nc.sync.dma_start(pooled_out[:], po[:])
    return nc


# ================= host wrapper =================

def kernel(**inputs):
    x = np.asarray(inputs["x"], np.float32)
    edge_index = np.asarray(inputs["edge_index"]).astype(np.int64)
    batch = np.asarray(inputs["batch"]).astype(np.int64)

    if "sch" not in _CACHE:
        _CACHE["sch"] = _build_schedule(edge_index)
        _CACHE["nc"] = _build_bass(_CACHE["sch"])
        _CACHE["nc"].compile()
    sch = _CACHE["sch"]
    nc = _CACHE["nc"]

    # ---- weight prep
    def a_tilde(a):  # [H, HID] -> [D, H] block diag
        m = np.zeros((D, H), np.float32)
        for h in range(H):
            m[h * HID:(h + 1) * HID, h] = a[h]
        return m

    rhs = []
    breps = []
    for l in range(3):
        W = np.asarray(inputs[f"conv{l}_w"], np.float32)
        a_s = np.asarray(inputs[f"conv{l}_as"], np.float32)
        a_d = np.asarray(inputs[f"conv{l}_ad"], np.float32)
        bb = np.asarray(inputs[f"conv{l}_b"], np.float32)
        rhs.append(np.concatenate(
            [W, W @ a_tilde(a_s), W @ a_tilde(a_d)], axis=1).astype(np.float16))
        breps.append(np.tile(bb[None, :], (P, 1)).astype(np.float16))

    eye = np.eye(P, dtype=np.float16)
    b1r = np.tile(np.asarray(inputs["enc_b1"], np.float32)[None, :], (P, 1))
    gr = np.tile(np.asarray(inputs["enc_g"], np.float32)[None, :], (P, 1))
    ber = np.tile(np.asarray(inputs["enc_be"], np.float32)[None, :], (P, 1))
    b2r = np.tile(np.asarray(inputs["enc_b2"], np.float32)[None, :], (P, 1))

    in_maps = []
    for c in range(NC):
        xc = np.zeros((NPAD, IN), np.float16)
        xc[:NPC] = x[c * NPC:(c + 1) * NPC].astype(np.float16)
        # bpool one-hot, [128, NB*G]: bp[p, b*G+g] = (batch[node b*128+p]==g)
        bp = np.zeros((P, NB * G), np.float16)
        bc = batch[c * NPC:(c + 1) * NPC]
        nodes = np.arange(NPC)
        bp[nodes % P, (nodes // P) * G + bc] = 1.0
        pc = sch["per_core"][c]
        in_maps.append({
            "xT": xc.T.copy(),
            "idxA": pc["idxA"], "idxB": pc["idxB"],
            "s0": pc["s0"], "s0t": pc["s0t"],
            "eye": eye,
            "encw1": np.asarray(inputs["enc_w1"], np.float16),
            "encw2": np.asarray(inputs["enc_w2"], np.float16),
            "b1r": b1r, "gr": gr, "ber": ber, "b2r": b2r,
            "rhs0": rhs[0], "rhs1": rhs[1], "rhs2": rhs[2],
            "brep0": breps[0], "brep1": breps[1], "brep2": breps[2],
            "bpool": bp,
        })

    LAST_RESULTS["in_maps"] = in_maps
    res = run_bass_kernel_spmd(nc, in_maps, core_ids=list(range(NC)),
                               trace=TRACE)
    LAST_RESULTS["res"] = res

    pooled = np.zeros((G, D), np.float32)
    for c in range(NC):
        pooled += res.results[c]["pooled"]
    cnt = np.bincount(batch, minlength=G).astype(np.float32)[:, None]
    pooled = pooled / np.maximum(cnt, 1.0)

    # decoder MLP on host (f32, matches reference ops)
    w1 = np.asarray(inputs["dec_w1"], np.float32)
    b1 = np.asarray(inputs["dec_b1"], np.float32)
    g_ = np.asarray(inputs["dec_g"], np.float32)
    be = np.asarray(inputs["dec_be"], np.float32)
    w2 = np.asarray(inputs["dec_w2"], np.float32)
    b2 = np.asarray(inputs["dec_b2"], np.float32)
    t = pooled @ w1 + b1
    m = t.mean(-1, keepdims=True)
    v = np.square(t - m).mean(-1, keepdims=True)
    t = g_ * (t - m) / np.sqrt(v + EPS) + be
    t = np.maximum(t, 0.0)
    out = t @ w2 + b2
    return out.astype(np.float32)


# revision 5
# speedup vs baseline: 1.7007x; 1.1853x over previous
"""Trainium2 Bass kernel for nn_GATSuper (3-layer GAT + encoder/decoder MLPs).

Strategy (8 NeuronCores, SPMD):
  - Nodes sharded: core c owns global nodes [c*6250, (c+1)*6250), padded to 6272.
  - Node feature table rows are 512B: [h' 256 fp8e4m3 | al_s 4 f16 | pad].
    fp8 features cut gather traffic 33% vs f16 and let the aggregation
    matmul run at fp8 rate.
  - The gathered table is built with TWO AllGathers per layer so each can
    overlap compute: table A = each core's node positions [0,3200) (rows
    owner*3200+pos, 25600 total), table B = positions [3200,6272) (rows
    owner*3072+(pos-3200), 24576 total). Both < 32768 so dma_gather's
    int16 indices work without further splitting. AG-A fires after the
    W phase of node blocks 0..24, AG-B after blocks 25..48.
  - Edges partitioned by dst owner; within a core, grouped per dst block
    (128 dst nodes), each block's slots = [A-half tiles | B-half tiles].
  - Per layer edge phase (per dst block): dma_gather rows per edge slot,
    per-edge al_d via one-hot S0T matmul, scores s=al_s+al_d, w =
    exp(leaky_relu(s)) (f16), w cast to fp8 in-row, features weighted by w
    per head (fp8 DVE), per-block one-hot S0 aggregation matmul (fp8) also
    yields softmax denominators; divide, bias, ELU in f16; result written
    to SBUF-resident h (f16). The next layer's W matmul for the block is
    emitted inline so AllGathers and W work hide inside the edge phase.
  - Global mean pool partials per core via one-hot matmul; host sums
    partials, divides by counts, runs the decoder MLP.
"""
import sys

import ml_dtypes
import numpy as np

sys.path.insert(0, "/opt/trn_rl_repo")

from concourse import bass, bacc, mybir, tile  # noqa: E402
from concourse.bass_utils import run_bass_kernel_spmd  # noqa: E402

# ---------------- problem constants (hardcoded) ----------------
N, E, IN, HID, H, OUT, G = 50000, 800000, 128, 64, 4, 40, 8
D = HID * H  # 256
NEG_SLOPE = 0.2
EPS = 1e-5
NC = 8          # cores
P = 128
NPC = N // NC   # 6250 real nodes per core
NB = 49         # dst blocks per core (ceil(6250/128))
NPAD = NB * P   # 6272 padded nodes per core
APOS = 3200     # node positions [0,APOS) -> table A
BPOS = NPAD - APOS  # 3072 positions -> table B
AROWS = NC * APOS   # 25600
BROWS = NC * BPOS   # 24576
ROWB = 512          # bytes (= fp8 elements) per table row
WCOL = 264          # fp8 col where w (fp8) is written per edge row
RHSW = WCOL + H     # 268: agg matmul rhs width

F32 = mybir.dt.float32
F16 = mybir.dt.float16
F8 = mybir.dt.float8e4
I16 = mybir.dt.int16
AFT = mybir.ActivationFunctionType
ALU = mybir.AluOpType

TRACE = False
LAST_RESULTS = {}

_CACHE = {}


# ================= host-side schedule =================

def _build_schedule(edge_index):
    """Partition edges; build per-core gather index / one-hot arrays."""
    src = np.concatenate([edge_index[0], np.arange(N, dtype=np.int64)])
    dst = np.concatenate([edge_index[1], np.arange(N, dtype=np.int64)])

    owner = dst // NPC
    blk = (dst % NPC) // P
    dloc = (dst % NPC) % P
    s_owner = src // NPC
    s_pos = src % NPC
    half = (s_pos >= APOS).astype(np.int64)
    tabidx = np.where(half == 0, s_owner * APOS + s_pos,
                      s_owner * BPOS + (s_pos - APOS))

    # key = ((owner*NB + blk)*2 + half); count per key
    key = ((owner * NB + blk) * 2 + half)
    nkeys = NC * NB * 2
    counts = np.bincount(key, minlength=nkeys).reshape(NC, NB, 2)

    # uniform tiles per (block, half) across cores
    T = np.ceil(counts.max(axis=0) / P).astype(np.int64)  # [NB, 2]
    T = np.maximum(T, 1)

    # global chunk order: per block b: A tiles then B tiles
    chunk_ranges = {}
    a_off = {}
    b_off = {}
    acc = accA = accB = 0
    for b in range(NB):
        chunk_ranges[(b, 0)] = (acc, int(T[b, 0]))
        acc += int(T[b, 0])
        chunk_ranges[(b, 1)] = (acc, int(T[b, 1]))
        acc += int(T[b, 1])
        a_off[b] = accA
        accA += int(T[b, 0]) * P
        b_off[b] = accB
        accB += int(T[b, 1]) * P
    Ttot = acc
    slots_tot = Ttot * P

    # per-core slot arrays
    order = np.lexsort((half, blk, owner))
    src_sorted = tabidx[order]
    dl_sorted = dloc[order]
    own_sorted = owner[order]
    blk_sorted = blk[order]
    half_sorted = half[order]

    k_sorted = ((own_sorted * NB + blk_sorted) * 2 + half_sorted)
    run_starts = np.searchsorted(k_sorted, np.arange(nkeys))
    run_ends = np.searchsorted(k_sorted, np.arange(nkeys) + 1)

    per_core = []
    for c in range(NC):
        slot_src = np.zeros(slots_tot, np.int16)
        slot_dl = np.full(slots_tot, P + 1, np.int64)  # pad -> no one-hot
        slot_half = np.zeros(slots_tot, np.int8)
        for b in range(NB):
            for h in (0, 1):
                kidx = (c * NB + b) * 2 + h
                s, e = run_starts[kidx], run_ends[kidx]
                n = e - s
                c0, nt = chunk_ranges[(b, h)]
                off = c0 * P
                slot_src[off:off + n] = src_sorted[s:e]
                slot_dl[off:off + n] = dl_sorted[s:e]
                slot_half[off:off + nt * P] = h

        def wrap(a):
            return a.reshape(-1, 16).T.copy()  # [16, n/16]

        maskA = slot_half == 0
        idxA = np.tile(wrap(slot_src[maskA]), (8, 1))
        idxB = np.tile(wrap(slot_src[~maskA]), (8, 1))
        # one-hot S0 tiles: s0[t, e, d] = (dloc[slot]==d) fp8
        dl_i = slot_dl.reshape(Ttot, P)
        s0 = np.zeros((Ttot, P, P), ml_dtypes.float8_e4m3)
        tt, ee = np.nonzero(dl_i < P)
        s0[tt, ee, dl_i[tt, ee]] = 1.0
        s0_in = s0.transpose(1, 0, 2).reshape(P, Ttot * P).copy()
        s0t_in = s0.transpose(2, 0, 1).reshape(P, Ttot * P).copy()
        per_core.append(dict(idxA=idxA, idxB=idxB, s0=s0_in, s0t=s0t_in))

    return dict(T=T, chunk_ranges=chunk_ranges, Ttot=Ttot,
                a_off=a_off, b_off=b_off, slots_tot=slots_tot,
                nA=accA, nB=accB, per_core=per_core)


# ================= bass program =================

def _build_bass(sch):
    T = sch["T"]
    chunk_ranges = sch["chunk_ranges"]
    Ttot = sch["Ttot"]
    nA, nB = sch["nA"], sch["nB"]

    nc = bacc.Bacc(None, target_bir_lowering=False, num_devices=NC,
                   num_swdge_queues=4)

    # ---- inputs
    xT = nc.dram_tensor("xT", [P, NPAD], F16, kind="ExternalInput")
    idxA = nc.dram_tensor("idxA", [P, nA // 16], I16, kind="ExternalInput")
    idxB = nc.dram_tensor("idxB", [P, nB // 16], I16, kind="ExternalInput")
    s0_in = nc.dram_tensor("s0", [P, Ttot * P], F8, kind="ExternalInput")
    s0t_in = nc.dram_tensor("s0t", [P, Ttot * P], F8, kind="ExternalInput")
    eye_in = nc.dram_tensor("eye", [P, P], F16, kind="ExternalInput")
    encw1 = nc.dram_tensor("encw1", [IN, HID], F16, kind="ExternalInput")
    encw2 = nc.dram_tensor("encw2", [HID, HID], F16, kind="ExternalInput")
    b1r_in = nc.dram_tensor("b1r", [P, HID], F32, kind="ExternalInput")
    gr_in = nc.dram_tensor("gr", [P, HID], F32, kind="ExternalInput")
    ber_in = nc.dram_tensor("ber", [P, HID], F32, kind="ExternalInput")
    b2r_in = nc.dram_tensor("b2r", [P, HID], F32, kind="ExternalInput")
    rhs_in = [nc.dram_tensor(f"rhs{l}", [HID if l == 0 else D, D + 2 * H],
                             F16, kind="ExternalInput") for l in range(3)]
    brep_in = [nc.dram_tensor(f"brep{l}", [P, D], F16, kind="ExternalInput")
               for l in range(3)]
    bpool_in = nc.dram_tensor("bpool", [P, NB * G], F16, kind="ExternalInput")

    pooled_out = nc.dram_tensor("pooled", [G, D], F32, kind="ExternalOutput")

    with tile.TileContext(nc) as tc:
        with tc.tile_pool(name="const", bufs=1) as cst, \
             tc.tile_pool(name="hwork", bufs=3) as hwork, \
             tc.tile_pool(name="lhsT", bufs=3) as lhsp, \
             tc.tile_pool(name="hcat", bufs=3) as hcatp, \
             tc.tile_pool(name="gbuf", bufs=6) as gbuf, \
             tc.tile_pool(name="s0b", bufs=6) as s0buf, \
             tc.tile_pool(name="small", bufs=4) as smallp, \
             tc.tile_pool(name="outp", bufs=3) as outp, \
             tc.tile_pool(name="pt", bufs=1, space="PSUM") as pt, \
             tc.tile_pool(name="pw", bufs=2, space="PSUM") as pw, \
             tc.tile_pool(name="pe", bufs=3, space="PSUM") as pep, \
             tc.tile_pool(name="pad", bufs=1, space="PSUM") as pad, \
             tc.tile_pool(name="pp", bufs=1, space="PSUM") as ppool, \
             tc.tile_pool(name="dram", bufs=1, space="DRAM") as dram:

            # ---- load constants
            def load(t_in, shape, nm, dt=F32):
                t = cst.tile(shape, dt, name=nm)
                nc.sync.dma_start(t[:], t_in[:])
                return t

            idxA_t = load(idxA, [P, nA // 16], "idxA_t", I16)
            idxB_t = load(idxB, [P, nB // 16], "idxB_t", I16)
            eye_t = load(eye_in, [P, P], "eye_t", F16)
            encw1_t = load(encw1, [IN, HID], "encw1_t", F16)
            encw2_t = load(encw2, [HID, HID], "encw2_t", F16)
            b1r_t = load(b1r_in, [P, HID], "b1r_t")
            gr_t = load(gr_in, [P, HID], "gr_t")
            ber_t = load(ber_in, [P, HID], "ber_t")
            b2r_t = load(b2r_in, [P, HID], "b2r_t")
            bpool_t = load(bpool_in, [P, NB * G], "bpool_t", F16)
            rhs_t = []
            for l in range(3):
                if l == 0:
                    r0 = cst.tile([HID, D + 2 * H], F16, name=f"rhsL{l}")
                    nc.sync.dma_start(r0[:], rhs_in[l][:])
                    rhs_t.append([r0])
                else:
                    chunks = []
                    for cch in range(D // P):
                        rc = cst.tile([P, D + 2 * H], F16,
                                      name=f"rhsL{l}c{cch}")
                        nc.sync.dma_start(
                            rc[:], rhs_in[l][cch * P:(cch + 1) * P, :])
                        chunks.append(rc)
                    rhs_t.append(chunks)
            brep_t = [load(brep_in[l], [P, D], f"brep_t{l}", F16)
                      for l in range(3)]
            h0_t = cst.tile([P, NB * HID], F16)  # encoder out, SBUF-resident
            h_sb = cst.tile([P, NB * D], F16)    # GAT layer io, SBUF-resident
            ald_t = cst.tile([P, NB * H], F16)   # per-layer al_d

            # ---- DRAM scratch
            hcat_own_l = [dram.tile([NPAD, ROWB], F8, name=f"hcown{l}")
                          for l in range(3)]
            hcatA_l = [dram.tile([AROWS, ROWB], F8, name=f"hcA{l}",
                                 addr_space="Shared") for l in range(3)]
            hcatB_l = [dram.tile([BROWS, ROWB], F8, name=f"hcB{l}",
                                 addr_space="Shared") for l in range(3)]

            def emit_ag(l, half):
                own = hcat_own_l[l]
                if half == 0:
                    nc.gpsimd.collective_compute(
                        "AllGather", ALU.bypass,
                        replica_groups=[list(range(NC))],
                        ins=[own[0:APOS, :].opt()],
                        outs=[hcatA_l[l][:].opt()])
                else:
                    nc.gpsimd.collective_compute(
                        "AllGather", ALU.bypass,
                        replica_groups=[list(range(NC))],
                        ins=[own[APOS:NPAD, :].opt()],
                        outs=[hcatB_l[l][:].opt()])

            def emit_w_block(l, n, rhs_chunks):
                """W matmul for layer l, node block n -> hcat_own[l] rows."""
                psw = pw.tile([P, D + 2 * H], F32, space="PSUM", tag="pw")
                if l == 0:
                    tin = h0_t[:, n * HID:(n + 1) * HID]
                    pst = pt.tile([HID, P], F16, space="PSUM", tag="pt")
                    lt = lhsp.tile([HID, P], F16, tag="lt64")
                    nc.tensor.transpose(pst[:], tin, eye_t[:])
                    nc.scalar.activation(lt[:], pst[:], AFT.Copy)
                    nc.tensor.matmul(psw[:], lhsT=lt[:], rhs=rhs_chunks[0][:],
                                     start=True, stop=True)
                else:
                    for cch in range(2):
                        tin = h_sb[:, n * D + cch * P:n * D + (cch + 1) * P]
                        pst = pt.tile([P, P], F16, space="PSUM", tag="pt")
                        lt = lhsp.tile([P, P], F16, tag="lt128")
                        nc.tensor.transpose(pst[:], tin, eye_t[:])
                        nc.scalar.activation(lt[:], pst[:], AFT.Copy)
                        nc.tensor.matmul(psw[:], lhsT=lt[:],
                                         rhs=rhs_chunks[cch][:],
                                         start=(cch == 0), stop=(cch == 1))
                hct = hcatp.tile([P, ROWB], F8, tag="hc")
                nc.scalar.activation(hct[:, 0:D], psw[:, 0:D], AFT.Copy)
                hct16 = hct[:].bitcast(F16)
                nc.scalar.activation(hct16[:, D // 2:D // 2 + H],
                                     psw[:, D:D + H], AFT.Copy)
                nc.scalar.activation(ald_t[:, n * H:(n + 1) * H],
                                     psw[:, D + H:D + 2 * H], AFT.Copy)
                nc.sync.dma_start(hcat_own_l[l][n * P:(n + 1) * P, :], hct[:])

            # ================ encoder + layer-0 W phase ================
            for n in range(NB):
                xt_blk = hwork.tile([P, P], F16, tag="xt")
                nc.sync.dma_start(xt_blk[:], xT[:, n * P:(n + 1) * P])
                psum1 = pw.tile([P, HID], F32, space="PSUM", tag="pw")
                nc.tensor.matmul(psum1[:], lhsT=xt_blk[:],
                                 rhs=encw1_t[:], start=True, stop=True)
                t = hwork.tile([P, HID], F32, tag="enc")
                nc.vector.tensor_tensor(out=t[:], in0=psum1[:], in1=b1r_t[:],
                                        op=ALU.add)
                # layernorm over HID
                mean = smallp.tile([P, 1], F32, tag="m")
                nc.vector.reduce_sum(out=mean[:], in_=t[:],
                                     axis=mybir.AxisListType.X)
                nc.vector.tensor_scalar_mul(mean[:], mean[:], 1.0 / HID)
                nc.vector.tensor_scalar(out=t[:], in0=t[:], scalar1=mean[:],
                                        scalar2=None, op0=ALU.subtract)
                sq = hwork.tile([P, HID], F32, tag="sq")
                nc.scalar.square(sq[:], t[:])
                var = smallp.tile([P, 1], F32, tag="v")
                nc.vector.reduce_sum(out=var[:], in_=sq[:],
                                     axis=mybir.AxisListType.X)
                nc.vector.tensor_scalar(out=var[:], in0=var[:],
                                        scalar1=1.0 / HID, scalar2=EPS,
                                        op0=ALU.mult, op1=ALU.add)
                nc.scalar.sqrt(var[:], var[:])
                nc.vector.reciprocal(var[:], var[:])
                nc.vector.tensor_scalar(out=t[:], in0=t[:], scalar1=var[:],
                                        scalar2=None, op0=ALU.mult)
                nc.vector.tensor_tensor(out=t[:], in0=t[:], in1=gr_t[:],
                                        op=ALU.mult)
                nc.vector.tensor_tensor(out=t[:], in0=t[:], in1=ber_t[:],
                                        op=ALU.add)
                t16 = hwork.tile([P, HID], F16, tag="enc16")
                nc.scalar.activation(t16[:], t[:], AFT.Relu)
                pst = pt.tile([HID, P], F16, space="PSUM", tag="pt")
                nc.tensor.transpose(pst[:], t16[:], eye_t[:])
                lt = lhsp.tile([HID, P], F16, tag="lt64")
                nc.scalar.activation(lt[:], pst[:], AFT.Copy)
                psum2 = pw.tile([P, HID], F32, space="PSUM", tag="pw")
                nc.tensor.matmul(psum2[:], lhsT=lt[:], rhs=encw2_t[:],
                                 start=True, stop=True)
                nc.vector.tensor_tensor(out=h0_t[:, n * HID:(n + 1) * HID],
                                        in0=psum2[:], in1=b2r_t[:], op=ALU.add)
                emit_w_block(0, n, rhs_t[0])
                if n == 24:
                    emit_ag(0, 0)
            emit_ag(0, 1)

            # ================ GAT layers ================
            qn = [0]

            def next_q():
                qn[0] = (qn[0] + 1) % 4
                return qn[0]

            for l in range(3):
                psp = None
                if l == 2:
                    psp = ppool.tile([G, D], F32, space="PSUM", name="psp")
                for b in range(NB):
                    cA0, tA = chunk_ranges[(b, 0)]
                    cB0, tB = chunk_ranges[(b, 1)]
                    cap = tA + tB
                    g0 = cA0
                    gt = gbuf.tile([P, cap * ROWB], F8, tag="g")
                    g3 = gt[:].rearrange("p (r c) -> p r c", r=cap)
                    offA = sch["a_off"][b]
                    offB = sch["b_off"][b]
                    # split each half-gather in two so all 4 SWDGE queues
                    # (= 4 Q7 core pairs) generate descriptors in parallel
                    parts = []
                    tA1 = tA // 2
                    if tA1:
                        parts.append((hcatA_l[l], idxA_t, offA, 0, tA1))
                    parts.append((hcatA_l[l], idxA_t, offA + tA1 * P, tA1,
                                  tA - tA1))
                    tB1 = tB // 2
                    if tB1:
                        parts.append((hcatB_l[l], idxB_t, offB, tA, tB1))
                    parts.append((hcatB_l[l], idxB_t, offB + tB1 * P,
                                  tA + tB1, tB - tB1))
                    for src_t, idx_t, off, t0, nt in parts:
                        nc.gpsimd.dma_gather(
                            out_ap=g3[:, t0:t0 + nt, :],
                            in_ap=src_t[:],
                            idxs_ap=idx_t[:, off // 16:(off + nt * P) // 16],
                            num_idxs=nt * P, num_idxs_reg=nt * P,
                            elem_size=ROWB, single_packet=False,
                            queue_num=next_q())
                    # one-hot tiles for this block
                    s0g = s0buf.tile([P, cap * P], F8, tag="s0g")
                    nc.scalar.dma_start(s0g[:], s0_in[:, g0 * P:(g0 + cap) * P])
                    s0tg = s0buf.tile([P, cap * P], F8, tag="s0tg")
                    nc.sync.dma_start(s0tg[:], s0t_in[:, g0 * P:(g0 + cap) * P])
                    # per-edge al_d via PE
                    adp = pad.tile([P, cap * H], F32, space="PSUM", tag="adp")
                    for t_ in range(cap):
                        nc.tensor.matmul(
                            adp[:, t_ * H:(t_ + 1) * H],
                            lhsT=s0tg[:, t_ * P:(t_ + 1) * P],
                            rhs=ald_t[:, b * H:(b + 1) * H],
                            start=True, stop=True)
                    # scores: copy strided al_s to a compact buffer
                    # (scalar engine handles the strided access), then do
                    # the arithmetic on contiguous f16 on DVE
                    g16 = gt[:].bitcast(F16).rearrange(
                        "p (r c) -> p r c", r=cap)
                    sl = g16[:, :, D // 2:D // 2 + H]
                    sc = smallp.tile([P, cap * H], F16, tag="sc")
                    sc3 = sc[:].rearrange("p (r c) -> p r c", r=cap)
                    nc.scalar.activation(sc3, sl, AFT.Copy)
                    nc.vector.tensor_tensor(out=sc[:], in0=sc[:], in1=adp[:],
                                            op=ALU.add)
                    # leaky relu: max(s, 0.2*s)
                    tmp = smallp.tile([P, cap * H], F16, tag="lrl")
                    nc.vector.tensor_scalar_mul(tmp[:], sc[:], NEG_SLOPE)
                    nc.vector.tensor_tensor(out=sc[:], in0=sc[:], in1=tmp[:],
                                            op=ALU.max)
                    # w = exp(s), cast to fp8 into row col WCOL
                    nc.scalar.activation(sc[:], sc[:], AFT.Exp)
                    w8 = g3[:, :, WCOL:WCOL + H]
                    nc.scalar.activation(w8, sc3, AFT.Copy)
                    # weight features by w per head (fp8 in-place)
                    for hh in range(H):
                        in0 = g3[:, :, hh * HID:(hh + 1) * HID]
                        in1 = g3[:, :, WCOL + hh:WCOL + hh + 1].to_broadcast(
                            [P, cap, HID])
                        nc.vector.tensor_tensor(out=in0, in0=in0, in1=in1,
                                                op=ALU.mult)
                    # per-block aggregation matmul (fp8)
                    pse = pep.tile([P, RHSW], F32, space="PSUM", tag="pe")
                    for i in range(cap):
                        nc.tensor.matmul(pse[:],
                                         lhsT=s0g[:, i * P:(i + 1) * P],
                                         rhs=g3[:, i, 0:RHSW],
                                         start=(i == 0), stop=(i == cap - 1))
                    # softmax divide + bias + ELU (f16)
                    den = smallp.tile([P, H], F32, tag="den")
                    nc.vector.tensor_scalar(out=den[:],
                                            in0=pse[:, WCOL:WCOL + H],
                                            scalar1=1e-16, scalar2=None,
                                            op0=ALU.add)
                    nc.vector.reciprocal(den[:], den[:])
                    xo = outp.tile([P, D], F16, tag="xo")
                    den_b = den[:].rearrange(
                        "p (h o) -> p h o", o=1).to_broadcast([P, H, HID])
                    nc.vector.tensor_tensor(
                        out=xo[:].rearrange("p (h c) -> p h c", h=H),
                        in0=pse[:, 0:D].rearrange("p (h c) -> p h c", h=H),
                        in1=den_b, op=ALU.mult)
                    nc.vector.tensor_tensor(out=xo[:], in0=xo[:],
                                            in1=brep_t[l][:], op=ALU.add)
                    # ELU: (relu(x)-1) + exp(x-relu(x))
                    r = outp.tile([P, D], F16, tag="r")
                    nc.scalar.activation(r[:], xo[:], AFT.Relu)
                    m = outp.tile([P, D], F16, tag="m")
                    nc.vector.tensor_tensor(out=m[:], in0=xo[:], in1=r[:],
                                            op=ALU.subtract)
                    nc.scalar.activation(m[:], m[:], AFT.Exp)
                    nc.vector.tensor_scalar(out=r[:], in0=r[:], scalar1=-1.0,
                                            scalar2=None, op0=ALU.add)
                    hout = h_sb[:, b * D:(b + 1) * D]
                    nc.vector.tensor_tensor(out=hout, in0=r[:],
                                            in1=m[:], op=ALU.add)
                    if l < 2:
                        # next layer W phase for this block, inline
                        emit_w_block(l + 1, b, rhs_t[l + 1])
                        if b == 24:
                            emit_ag(l + 1, 0)
                        elif b == NB - 1:
                            emit_ag(l + 1, 1)
                    else:
                        # pooling partial: psp += Bp.T @ h
                        nc.tensor.matmul(
                            psp[:], lhsT=bpool_t[:, b * G:(b + 1) * G],
                            rhs=hout, start=(b == 0), stop=(b == NB - 1))
                        if b == NB - 1:
                            po = outp.tile([G, D], F32, tag="po")
                            nc.scalar.activation(po[:], psp[:], AFT.Copy)
                            nc.sync.dma_start(pooled_out[:], po[:])
    return nc


# ================= host wrapper =================

def kernel(**inputs):
    x = np.asarray(inputs["x"], np.float32)
    edge_index = np.asarray(inputs["edge_index"]).astype(np.int64)
    batch = np.asarray(inputs["batch"]).astype(np.int64)

    if "sch" not in _CACHE:
        _CACHE["sch"] = _build_schedule(edge_index)
        _CACHE["nc"] = _build_bass(_CACHE["sch"])
        _CACHE["nc"].compile()
    sch = _CACHE["sch"]
    nc = _CACHE["nc"]

    # ---- weight prep
    def a_tilde(a):  # [H, HID] -> [D, H] block diag
        m = np.zeros((D, H), np.float32)
        for h in range(H):
            m[h * HID:(h + 1) * HID, h] = a[h]
        return m

    rhs = []
    breps = []
    for l in range(3):
        W = np.asarray(inputs[f"conv{l}_w"], np.float32)
        a_s = np.asarray(inputs[f"conv{l}_as"], np.float32)
        a_d = np.asarray(inputs[f"conv{l}_ad"], np.float32)
        bb = np.asarray(inputs[f"conv{l}_b"], np.float32)
        rhs.append(np.concatenate(
            [W, W @ a_tilde(a_s), W @ a_tilde(a_d)], axis=1).astype(np.float16))
        breps.append(np.tile(bb[None, :], (P, 1)).astype(np.float16))

    eye = np.eye(P, dtype=np.float16)
    b1r = np.tile(np.asarray(inputs["enc_b1"], np.float32)[None, :], (P, 1))
    gr = np.tile(np.asarray(inputs["enc_g"], np.float32)[None, :], (P, 1))
    ber = np.tile(np.asarray(inputs["enc_be"], np.float32)[None, :], (P, 1))
    b2r = np.tile(np.asarray(inputs["enc_b2"], np.float32)[None, :], (P, 1))

    in_maps = []
    for c in range(NC):
        xc = np.zeros((NPAD, IN), np.float16)
        xc[:NPC] = x[c * NPC:(c + 1) * NPC].astype(np.float16)
        # bpool one-hot, [128, NB*G]: bp[p, b*G+g] = (batch[node b*128+p]==g)
        bp = np.zeros((P, NB * G), np.float16)
        bc = batch[c * NPC:(c + 1) * NPC]
        nodes = np.arange(NPC)
        bp[nodes % P, (nodes // P) * G + bc] = 1.0
        pc = sch["per_core"][c]
        in_maps.append({
            "xT": xc.T.copy(),
            "idxA": pc["idxA"], "idxB": pc["idxB"],
            "s0": pc["s0"], "s0t": pc["s0t"],
            "eye": eye,
            "encw1": np.asarray(inputs["enc_w1"], np.float16),
            "encw2": np.asarray(inputs["enc_w2"], np.float16),
            "b1r": b1r, "gr": gr, "ber": ber, "b2r": b2r,
            "rhs0": rhs[0], "rhs1": rhs[1], "rhs2": rhs[2],
            "brep0": breps[0], "brep1": breps[1], "brep2": breps[2],
            "bpool": bp,
        })

    LAST_RESULTS["in_maps"] = in_maps
    res = run_bass_kernel_spmd(nc, in_maps, core_ids=list(range(NC)),
                               trace=TRACE)
    LAST_RESULTS["res"] = res

    pooled = np.zeros((G, D), np.float32)
    for c in range(NC):
        pooled += res.results[c]["pooled"]
    cnt = np.bincount(batch, minlength=G).astype(np.float32)[:, None]
    pooled = pooled / np.maximum(cnt, 1.0)

    # decoder MLP on host (f32, matches reference ops)
    w1 = np.asarray(inputs["dec_w1"], np.float32)
    b1 = np.asarray(inputs["dec_b1"], np.float32)
    g_ = np.asarray(inputs["dec_g"], np.float32)
    be = np.asarray(inputs["dec_be"], np.float32)
    w2 = np.asarray(inputs["dec_w2"], np.float32)
    b2 = np.asarray(inputs["dec_b2"], np.float32)
    t = pooled @ w1 + b1
    m = t.mean(-1, keepdims=True)
    v = np.square(t - m).mean(-1, keepdims=True)
    t = g_ * (t - m) / np.sqrt(v + EPS) + be
    t = np.maximum(t, 0.0)
    out = t @ w2 + b2
    return out.astype(np.float32)


# revision 6
# speedup vs baseline: 1.7209x; 1.0119x over previous
"""Trainium2 Bass kernel for nn_GATSuper (3-layer GAT + encoder/decoder MLPs).

Strategy (8 NeuronCores, SPMD):
  - Nodes sharded: core c owns global nodes [c*6250, (c+1)*6250), padded to 6272.
  - Node feature table rows are 512B: [h' 256 fp8e4m3 | al_s 4 f16 | pad].
    fp8 features cut gather traffic 33% vs f16 and let the aggregation
    matmul run at fp8 rate.
  - The gathered table is built with TWO AllGathers per layer so each can
    overlap compute: table A = each core's node positions [0,3200) (rows
    owner*3200+pos, 25600 total), table B = positions [3200,6272) (rows
    owner*3072+(pos-3200), 24576 total). Both < 32768 so dma_gather's
    int16 indices work without further splitting. AG-A fires after the
    W phase of node blocks 0..24, AG-B after blocks 25..48.
  - Edges partitioned by dst owner; within a core, grouped per dst block
    (128 dst nodes), each block's slots = [A-half tiles | B-half tiles].
  - Per layer edge phase (per dst block): dma_gather rows per edge slot,
    per-edge al_d via one-hot S0T matmul, scores s=al_s+al_d, w =
    exp(leaky_relu(s)) (f16), w cast to fp8 in-row, features weighted by w
    per head (fp8 DVE), per-block one-hot S0 aggregation matmul (fp8) also
    yields softmax denominators; divide, bias, ELU in f16; result written
    to SBUF-resident h (f16). The next layer's W matmul for the block is
    emitted inline so AllGathers and W work hide inside the edge phase.
  - Global mean pool partials per core via one-hot matmul; host sums
    partials, divides by counts, runs the decoder MLP.
"""
import sys

import ml_dtypes
import numpy as np

sys.path.insert(0, "/opt/trn_rl_repo")

from concourse import bass, bacc, mybir, tile  # noqa: E402
from concourse.bass_utils import run_bass_kernel_spmd  # noqa: E402

# ---------------- problem constants (hardcoded) ----------------
N, E, IN, HID, H, OUT, G = 50000, 800000, 128, 64, 4, 40, 8
D = HID * H  # 256
NEG_SLOPE = 0.2
EPS = 1e-5
NC = 8          # cores
P = 128
NPC = N // NC   # 6250 real nodes per core
NB = 49         # dst blocks per core (ceil(6250/128))
NPAD = NB * P   # 6272 padded nodes per core
APOS = 3200     # node positions [0,APOS) -> table A
BPOS = NPAD - APOS  # 3072 positions -> table B
AROWS = NC * APOS   # 25600
BROWS = NC * BPOS   # 24576
ROWB = 512          # bytes (= fp8 elements) per table row
WCOL = 264          # fp8 col where w (fp8) is written per edge row
RHSW = WCOL + H     # 268: agg matmul rhs width

F32 = mybir.dt.float32
F16 = mybir.dt.float16
F8 = mybir.dt.float8e4
I16 = mybir.dt.int16
AFT = mybir.ActivationFunctionType
ALU = mybir.AluOpType

TRACE = False
LAST_RESULTS = {}

_CACHE = {}


# ================= host-side schedule =================

def _build_schedule(edge_index):
    """Partition edges; build per-core gather index / one-hot arrays."""
    src = np.concatenate([edge_index[0], np.arange(N, dtype=np.int64)])
    dst = np.concatenate([edge_index[1], np.arange(N, dtype=np.int64)])

    owner = dst // NPC
    blk = (dst % NPC) // P
    dloc = (dst % NPC) % P
    s_owner = src // NPC
    s_pos = src % NPC
    half = (s_pos >= APOS).astype(np.int64)
    tabidx = np.where(half == 0, s_owner * APOS + s_pos,
                      s_owner * BPOS + (s_pos - APOS))

    # key = ((owner*NB + blk)*2 + half); count per key
    key = ((owner * NB + blk) * 2 + half)
    nkeys = NC * NB * 2
    counts = np.bincount(key, minlength=nkeys).reshape(NC, NB, 2)

    # uniform tiles per (block, half) across cores
    T = np.ceil(counts.max(axis=0) / P).astype(np.int64)  # [NB, 2]
    T = np.maximum(T, 1)

    # global chunk order: per block b: A tiles then B tiles
    chunk_ranges = {}
    a_off = {}
    b_off = {}
    acc = accA = accB = 0
    for b in range(NB):
        chunk_ranges[(b, 0)] = (acc, int(T[b, 0]))
        acc += int(T[b, 0])
        chunk_ranges[(b, 1)] = (acc, int(T[b, 1]))
        acc += int(T[b, 1])
        a_off[b] = accA
        accA += int(T[b, 0]) * P
        b_off[b] = accB
        accB += int(T[b, 1]) * P
    Ttot = acc
    slots_tot = Ttot * P

    # per-core slot arrays
    order = np.lexsort((half, blk, owner))
    src_sorted = tabidx[order]
    dl_sorted = dloc[order]
    own_sorted = owner[order]
    blk_sorted = blk[order]
    half_sorted = half[order]

    k_sorted = ((own_sorted * NB + blk_sorted) * 2 + half_sorted)
    run_starts = np.searchsorted(k_sorted, np.arange(nkeys))
    run_ends = np.searchsorted(k_sorted, np.arange(nkeys) + 1)

    per_core = []
    for c in range(NC):
        slot_src = np.zeros(slots_tot, np.int16)
        slot_dl = np.full(slots_tot, P + 1, np.int64)  # pad -> no one-hot
        slot_half = np.zeros(slots_tot, np.int8)
        for b in range(NB):
            for h in (0, 1):
                kidx = (c * NB + b) * 2 + h
                s, e = run_starts[kidx], run_ends[kidx]
                n = e - s
                c0, nt = chunk_ranges[(b, h)]
                off = c0 * P
                slot_src[off:off + n] = src_sorted[s:e]
                slot_dl[off:off + n] = dl_sorted[s:e]
                slot_half[off:off + nt * P] = h

        def wrap(a):
            return a.reshape(-1, 16).T.copy()  # [16, n/16]

        maskA = slot_half == 0
        idxA = np.tile(wrap(slot_src[maskA]), (8, 1))
        idxB = np.tile(wrap(slot_src[~maskA]), (8, 1))
        # one-hot S0 tiles: s0[t, e, d] = (dloc[slot]==d) fp8
        dl_i = slot_dl.reshape(Ttot, P)
        s0 = np.zeros((Ttot, P, P), ml_dtypes.float8_e4m3)
        tt, ee = np.nonzero(dl_i < P)
        s0[tt, ee, dl_i[tt, ee]] = 1.0
        s0_in = s0.transpose(1, 0, 2).reshape(P, Ttot * P).copy()
        s0t_in = s0.transpose(2, 0, 1).reshape(P, Ttot * P).copy()
        per_core.append(dict(idxA=idxA, idxB=idxB, s0=s0_in, s0t=s0t_in))

    return dict(T=T, chunk_ranges=chunk_ranges, Ttot=Ttot,
                a_off=a_off, b_off=b_off, slots_tot=slots_tot,
                nA=accA, nB=accB, per_core=per_core)


# ================= bass program =================

def _build_bass(sch):
    T = sch["T"]
    chunk_ranges = sch["chunk_ranges"]
    Ttot = sch["Ttot"]
    nA, nB = sch["nA"], sch["nB"]

    nc = bacc.Bacc(None, target_bir_lowering=False, num_devices=NC,
                   num_swdge_queues=4)

    # ---- inputs
    xT = nc.dram_tensor("xT", [P, NPAD], F16, kind="ExternalInput")
    idxA = nc.dram_tensor("idxA", [P, nA // 16], I16, kind="ExternalInput")
    idxB = nc.dram_tensor("idxB", [P, nB // 16], I16, kind="ExternalInput")
    s0_in = nc.dram_tensor("s0", [P, Ttot * P], F8, kind="ExternalInput")
    s0t_in = nc.dram_tensor("s0t", [P, Ttot * P], F8, kind="ExternalInput")
    eye_in = nc.dram_tensor("eye", [P, P], F16, kind="ExternalInput")
    encw1 = nc.dram_tensor("encw1", [IN, HID], F16, kind="ExternalInput")
    encw2 = nc.dram_tensor("encw2", [HID, HID], F16, kind="ExternalInput")
    b1r_in = nc.dram_tensor("b1r", [P, HID], F32, kind="ExternalInput")
    gr_in = nc.dram_tensor("gr", [P, HID], F32, kind="ExternalInput")
    ber_in = nc.dram_tensor("ber", [P, HID], F32, kind="ExternalInput")
    b2r_in = nc.dram_tensor("b2r", [P, HID], F32, kind="ExternalInput")
    rhs_in = [nc.dram_tensor(f"rhs{l}", [HID if l == 0 else D, D + 2 * H],
                             F16, kind="ExternalInput") for l in range(3)]
    brep_in = [nc.dram_tensor(f"brep{l}", [P, D], F16, kind="ExternalInput")
               for l in range(3)]
    bpool_in = nc.dram_tensor("bpool", [P, NB * G], F16, kind="ExternalInput")

    pooled_out = nc.dram_tensor("pooled", [G, D], F32, kind="ExternalOutput")

    with tile.TileContext(nc) as tc:
        with tc.tile_pool(name="const", bufs=1) as cst, \
             tc.tile_pool(name="hwork", bufs=3) as hwork, \
             tc.tile_pool(name="lhsT", bufs=3) as lhsp, \
             tc.tile_pool(name="hcat", bufs=3) as hcatp, \
             tc.tile_pool(name="gbuf", bufs=6) as gbuf, \
             tc.tile_pool(name="s0b", bufs=6) as s0buf, \
             tc.tile_pool(name="small", bufs=4) as smallp, \
             tc.tile_pool(name="outp", bufs=3) as outp, \
             tc.tile_pool(name="pt", bufs=1, space="PSUM") as pt, \
             tc.tile_pool(name="pw", bufs=2, space="PSUM") as pw, \
             tc.tile_pool(name="pe", bufs=3, space="PSUM") as pep, \
             tc.tile_pool(name="pad", bufs=1, space="PSUM") as pad, \
             tc.tile_pool(name="pp", bufs=1, space="PSUM") as ppool, \
             tc.tile_pool(name="dram", bufs=1, space="DRAM") as dram:

            # ---- load constants
            def load(t_in, shape, nm, dt=F32):
                t = cst.tile(shape, dt, name=nm)
                nc.sync.dma_start(t[:], t_in[:])
                return t

            idxA_t = load(idxA, [P, nA // 16], "idxA_t", I16)
            idxB_t = load(idxB, [P, nB // 16], "idxB_t", I16)
            eye_t = load(eye_in, [P, P], "eye_t", F16)
            encw1_t = load(encw1, [IN, HID], "encw1_t", F16)
            encw2_t = load(encw2, [HID, HID], "encw2_t", F16)
            b1r_t = load(b1r_in, [P, HID], "b1r_t")
            gr_t = load(gr_in, [P, HID], "gr_t")
            ber_t = load(ber_in, [P, HID], "ber_t")
            b2r_t = load(b2r_in, [P, HID], "b2r_t")
            bpool_t = load(bpool_in, [P, NB * G], "bpool_t", F16)
            rhs_t = []
            for l in range(3):
                if l == 0:
                    r0 = cst.tile([HID, D + 2 * H], F16, name=f"rhsL{l}")
                    nc.sync.dma_start(r0[:], rhs_in[l][:])
                    rhs_t.append([r0])
                else:
                    chunks = []
                    for cch in range(D // P):
                        rc = cst.tile([P, D + 2 * H], F16,
                                      name=f"rhsL{l}c{cch}")
                        nc.sync.dma_start(
                            rc[:], rhs_in[l][cch * P:(cch + 1) * P, :])
                        chunks.append(rc)
                    rhs_t.append(chunks)
            brep_t = [load(brep_in[l], [P, D], f"brep_t{l}", F16)
                      for l in range(3)]
            h0_t = cst.tile([P, NB * HID], F16)  # encoder out, SBUF-resident
            h_sb = cst.tile([P, NB * D], F16)    # GAT layer io, SBUF-resident
            ald_t = cst.tile([P, NB * H], F16)   # per-layer al_d

            # ---- DRAM scratch
            hcat_own_l = [dram.tile([NPAD, ROWB], F8, name=f"hcown{l}")
                          for l in range(3)]
            hcatA_l = [dram.tile([AROWS, ROWB], F8, name=f"hcA{l}",
                                 addr_space="Shared") for l in range(3)]
            hcatB_l = [dram.tile([BROWS, ROWB], F8, name=f"hcB{l}",
                                 addr_space="Shared") for l in range(3)]

            def emit_ag(l, half):
                own = hcat_own_l[l]
                if half == 0:
                    nc.gpsimd.collective_compute(
                        "AllGather", ALU.bypass,
                        replica_groups=[list(range(NC))],
                        ins=[own[0:APOS, :].opt()],
                        outs=[hcatA_l[l][:].opt()])
                else:
                    nc.gpsimd.collective_compute(
                        "AllGather", ALU.bypass,
                        replica_groups=[list(range(NC))],
                        ins=[own[APOS:NPAD, :].opt()],
                        outs=[hcatB_l[l][:].opt()])

            def emit_w_block(l, n, rhs_chunks):
                """W matmul for layer l, node block n -> hcat_own[l] rows."""
                psw = pw.tile([P, D + 2 * H], F32, space="PSUM", tag="pw")
                if l == 0:
                    tin = h0_t[:, n * HID:(n + 1) * HID]
                    pst = pt.tile([HID, P], F16, space="PSUM", tag="pt")
                    lt = lhsp.tile([HID, P], F16, tag="lt64")
                    nc.tensor.transpose(pst[:], tin, eye_t[:])
                    nc.scalar.activation(lt[:], pst[:], AFT.Copy)
                    nc.tensor.matmul(psw[:], lhsT=lt[:], rhs=rhs_chunks[0][:],
                                     start=True, stop=True)
                else:
                    for cch in range(2):
                        tin = h_sb[:, n * D + cch * P:n * D + (cch + 1) * P]
                        pst = pt.tile([P, P], F16, space="PSUM", tag="pt")
                        lt = lhsp.tile([P, P], F16, tag="lt128")
                        nc.tensor.transpose(pst[:], tin, eye_t[:])
                        nc.scalar.activation(lt[:], pst[:], AFT.Copy)
                        nc.tensor.matmul(psw[:], lhsT=lt[:],
                                         rhs=rhs_chunks[cch][:],
                                         start=(cch == 0), stop=(cch == 1))
                hct = hcatp.tile([P, ROWB], F8, tag="hc")
                nc.scalar.activation(hct[:, 0:D], psw[:, 0:D], AFT.Copy)
                hct16 = hct[:].bitcast(F16)
                nc.scalar.activation(hct16[:, D // 2:D // 2 + H],
                                     psw[:, D:D + H], AFT.Copy)
                nc.scalar.activation(ald_t[:, n * H:(n + 1) * H],
                                     psw[:, D + H:D + 2 * H], AFT.Copy)
                nc.sync.dma_start(hcat_own_l[l][n * P:(n + 1) * P, :], hct[:])

            # ================ encoder + layer-0 W phase ================
            for n in range(NB):
                xt_blk = hwork.tile([P, P], F16, tag="xt")
                nc.sync.dma_start(xt_blk[:], xT[:, n * P:(n + 1) * P])
                psum1 = pw.tile([P, HID], F32, space="PSUM", tag="pw")
                nc.tensor.matmul(psum1[:], lhsT=xt_blk[:],
                                 rhs=encw1_t[:], start=True, stop=True)
                t = hwork.tile([P, HID], F32, tag="enc")
                nc.vector.tensor_tensor(out=t[:], in0=psum1[:], in1=b1r_t[:],
                                        op=ALU.add)
                # layernorm over HID
                mean = smallp.tile([P, 1], F32, tag="m")
                nc.vector.reduce_sum(out=mean[:], in_=t[:],
                                     axis=mybir.AxisListType.X)
                nc.vector.tensor_scalar_mul(mean[:], mean[:], 1.0 / HID)
                nc.vector.tensor_scalar(out=t[:], in0=t[:], scalar1=mean[:],
                                        scalar2=None, op0=ALU.subtract)
                sq = hwork.tile([P, HID], F32, tag="sq")
                nc.scalar.square(sq[:], t[:])
                var = smallp.tile([P, 1], F32, tag="v")
                nc.vector.reduce_sum(out=var[:], in_=sq[:],
                                     axis=mybir.AxisListType.X)
                nc.vector.tensor_scalar(out=var[:], in0=var[:],
                                        scalar1=1.0 / HID, scalar2=EPS,
                                        op0=ALU.mult, op1=ALU.add)
                nc.scalar.sqrt(var[:], var[:])
                nc.vector.reciprocal(var[:], var[:])
                nc.vector.tensor_scalar(out=t[:], in0=t[:], scalar1=var[:],
                                        scalar2=None, op0=ALU.mult)
                nc.vector.tensor_tensor(out=t[:], in0=t[:], in1=gr_t[:],
                                        op=ALU.mult)
                nc.vector.tensor_tensor(out=t[:], in0=t[:], in1=ber_t[:],
                                        op=ALU.add)
                t16 = hwork.tile([P, HID], F16, tag="enc16")
                nc.scalar.activation(t16[:], t[:], AFT.Relu)
                pst = pt.tile([HID, P], F16, space="PSUM", tag="pt")
                nc.tensor.transpose(pst[:], t16[:], eye_t[:])
                lt = lhsp.tile([HID, P], F16, tag="lt64")
                nc.scalar.activation(lt[:], pst[:], AFT.Copy)
                psum2 = pw.tile([P, HID], F32, space="PSUM", tag="pw")
                nc.tensor.matmul(psum2[:], lhsT=lt[:], rhs=encw2_t[:],
                                 start=True, stop=True)
                nc.vector.tensor_tensor(out=h0_t[:, n * HID:(n + 1) * HID],
                                        in0=psum2[:], in1=b2r_t[:], op=ALU.add)
                emit_w_block(0, n, rhs_t[0])
                if n == 24:
                    emit_ag(0, 0)
            emit_ag(0, 1)

            # ================ GAT layers ================
            qn = [0]

            def next_q():
                qn[0] = (qn[0] + 1) % 4
                return qn[0]

            # Two-stage software pipeline per layer: stage 1 (gather, scores,
            # weighting, aggregation matmuls) for block b+1 is emitted BEFORE
            # stage 2 (softmax divide, ELU, next-layer W) of block b, so each
            # in-order engine queue interleaves two blocks' work instead of
            # serializing on the full per-block dependency chain.
            def edge_stage1(l, b):
                cA0, tA = chunk_ranges[(b, 0)]
                cB0, tB = chunk_ranges[(b, 1)]
                cap = tA + tB
                g0 = cA0
                gt = gbuf.tile([P, cap * ROWB], F8, tag="g")
                g3 = gt[:].rearrange("p (r c) -> p r c", r=cap)
                offA = sch["a_off"][b]
                offB = sch["b_off"][b]
                # split each half-gather in two so all 4 SWDGE queues
                # (= 4 Q7 core pairs) generate descriptors in parallel
                parts = []
                tA1 = tA // 2
                if tA1:
                    parts.append((hcatA_l[l], idxA_t, offA, 0, tA1))
                parts.append((hcatA_l[l], idxA_t, offA + tA1 * P, tA1,
                              tA - tA1))
                tB1 = tB // 2
                if tB1:
                    parts.append((hcatB_l[l], idxB_t, offB, tA, tB1))
                parts.append((hcatB_l[l], idxB_t, offB + tB1 * P,
                              tA + tB1, tB - tB1))
                for src_t, idx_t, off, t0, nt in parts:
                    nc.gpsimd.dma_gather(
                        out_ap=g3[:, t0:t0 + nt, :],
                        in_ap=src_t[:],
                        idxs_ap=idx_t[:, off // 16:(off + nt * P) // 16],
                        num_idxs=nt * P, num_idxs_reg=nt * P,
                        elem_size=ROWB, single_packet=False,
                        queue_num=next_q())
                # one-hot tiles for this block
                s0g = s0buf.tile([P, cap * P], F8, tag="s0g")
                nc.scalar.dma_start(s0g[:], s0_in[:, g0 * P:(g0 + cap) * P])
                s0tg = s0buf.tile([P, cap * P], F8, tag="s0tg")
                nc.sync.dma_start(s0tg[:], s0t_in[:, g0 * P:(g0 + cap) * P])
                # per-edge al_d via PE
                adp = pad.tile([P, cap * H], F32, space="PSUM", tag="adp")
                for t_ in range(cap):
                    nc.tensor.matmul(
                        adp[:, t_ * H:(t_ + 1) * H],
                        lhsT=s0tg[:, t_ * P:(t_ + 1) * P],
                        rhs=ald_t[:, b * H:(b + 1) * H],
                        start=True, stop=True)
                # scores: copy strided al_s to a compact buffer (scalar
                # engine handles the strided access), then contiguous DVE
                g16 = gt[:].bitcast(F16).rearrange("p (r c) -> p r c", r=cap)
                sl = g16[:, :, D // 2:D // 2 + H]
                sc = smallp.tile([P, cap * H], F16, tag="sc")
                sc3 = sc[:].rearrange("p (r c) -> p r c", r=cap)
                nc.scalar.activation(sc3, sl, AFT.Copy)
                nc.vector.tensor_tensor(out=sc[:], in0=sc[:], in1=adp[:],
                                        op=ALU.add)
                # leaky relu: max(s, 0.2*s)
                tmp = smallp.tile([P, cap * H], F16, tag="lrl")
                nc.vector.tensor_scalar_mul(tmp[:], sc[:], NEG_SLOPE)
                nc.vector.tensor_tensor(out=sc[:], in0=sc[:], in1=tmp[:],
                                        op=ALU.max)
                # w = exp(s), cast to fp8 into row col WCOL
                nc.scalar.activation(sc[:], sc[:], AFT.Exp)
                w8 = g3[:, :, WCOL:WCOL + H]
                nc.scalar.activation(w8, sc3, AFT.Copy)
                # weight features by w per head (fp8 in-place)
                for hh in range(H):
                    in0 = g3[:, :, hh * HID:(hh + 1) * HID]
                    in1 = g3[:, :, WCOL + hh:WCOL + hh + 1].to_broadcast(
                        [P, cap, HID])
                    nc.vector.tensor_tensor(out=in0, in0=in0, in1=in1,
                                            op=ALU.mult)
                # per-block aggregation matmul (fp8)
                pse = pep.tile([P, RHSW], F32, space="PSUM", tag="pe")
                for i in range(cap):
                    nc.tensor.matmul(pse[:],
                                     lhsT=s0g[:, i * P:(i + 1) * P],
                                     rhs=g3[:, i, 0:RHSW],
                                     start=(i == 0), stop=(i == cap - 1))
                return pse

            def edge_stage2(l, b, pse, psp):
                # softmax divide + bias + ELU (f16)
                den = smallp.tile([P, H], F32, tag="den")
                nc.vector.tensor_scalar(out=den[:],
                                        in0=pse[:, WCOL:WCOL + H],
                                        scalar1=1e-16, scalar2=None,
                                        op0=ALU.add)
                nc.vector.reciprocal(den[:], den[:])
                xo = outp.tile([P, D], F16, tag="xo")
                den_b = den[:].rearrange(
                    "p (h o) -> p h o", o=1).to_broadcast([P, H, HID])
                nc.vector.tensor_tensor(
                    out=xo[:].rearrange("p (h c) -> p h c", h=H),
                    in0=pse[:, 0:D].rearrange("p (h c) -> p h c", h=H),
                    in1=den_b, op=ALU.mult)
                nc.vector.tensor_tensor(out=xo[:], in0=xo[:],
                                        in1=brep_t[l][:], op=ALU.add)
                # ELU: (relu(x)-1) + exp(x-relu(x))
                r = outp.tile([P, D], F16, tag="r")
                nc.scalar.activation(r[:], xo[:], AFT.Relu)
                m = outp.tile([P, D], F16, tag="m")
                nc.vector.tensor_tensor(out=m[:], in0=xo[:], in1=r[:],
                                        op=ALU.subtract)
                nc.scalar.activation(m[:], m[:], AFT.Exp)
                nc.vector.tensor_scalar(out=r[:], in0=r[:], scalar1=-1.0,
                                        scalar2=None, op0=ALU.add)
                hout = h_sb[:, b * D:(b + 1) * D]
                nc.vector.tensor_tensor(out=hout, in0=r[:],
                                        in1=m[:], op=ALU.add)
                if l < 2:
                    # next layer W phase for this block, inline
                    emit_w_block(l + 1, b, rhs_t[l + 1])
                    if b == 24:
                        emit_ag(l + 1, 0)
                    elif b == NB - 1:
                        emit_ag(l + 1, 1)
                else:
                    # pooling partial: psp += Bp.T @ h
                    nc.tensor.matmul(
                        psp[:], lhsT=bpool_t[:, b * G:(b + 1) * G],
                        rhs=hout, start=(b == 0), stop=(b == NB - 1))
                    if b == NB - 1:
                        po = outp.tile([G, D], F32, tag="po")
                        nc.scalar.activation(po[:], psp[:], AFT.Copy)
                        nc.sync.dma_start(pooled_out[:], po[:])

            for l in range(3):
                psp = None
                if l == 2:
                    psp = ppool.tile([G, D], F32, space="PSUM", name="psp")
                prev = edge_stage1(l, 0)
                for b in range(1, NB):
                    cur = edge_stage1(l, b)
                    edge_stage2(l, b - 1, prev, psp)
                    prev = cur
                edge_stage2(l, NB - 1, prev, psp)
    return nc


# ================= host wrapper =================

def kernel(**inputs):
    x = np.asarray(inputs["x"], np.float32)
    edge_index = np.asarray(inputs["edge_index"]).astype(np.int64)
    batch = np.asarray(inputs["batch"]).astype(np.int64)

    if "sch" not in _CACHE:
        _CACHE["sch"] = _build_schedule(edge_index)
        _CACHE["nc"] = _build_bass(_CACHE["sch"])
        _CACHE["nc"].compile()
    sch = _CACHE["sch"]
    nc = _CACHE["nc"]

    # ---- weight prep
    def a_tilde(a):  # [H, HID] -> [D, H] block diag
        m = np.zeros((D, H), np.float32)
        for h in range(H):
            m[h * HID:(h + 1) * HID, h] = a[h]
        return m

    rhs = []
    breps = []
    for l in range(3):
        W = np.asarray(inputs[f"conv{l}_w"], np.float32)
        a_s = np.asarray(inputs[f"conv{l}_as"], np.float32)
        a_d = np.asarray(inputs[f"conv{l}_ad"], np.float32)
        bb = np.asarray(inputs[f"conv{l}_b"], np.float32)
        rhs.append(np.concatenate(
            [W, W @ a_tilde(a_s), W @ a_tilde(a_d)], axis=1).astype(np.float16))
        breps.append(np.tile(bb[None, :], (P, 1)).astype(np.float16))

    eye = np.eye(P, dtype=np.float16)
    b1r = np.tile(np.asarray(inputs["enc_b1"], np.float32)[None, :], (P, 1))
    gr = np.tile(np.asarray(inputs["enc_g"], np.float32)[None, :], (P, 1))
    ber = np.tile(np.asarray(inputs["enc_be"], np.float32)[None, :], (P, 1))
    b2r = np.tile(np.asarray(inputs["enc_b2"], np.float32)[None, :], (P, 1))

    in_maps = []
    for c in range(NC):
        xc = np.zeros((NPAD, IN), np.float16)
        xc[:NPC] = x[c * NPC:(c + 1) * NPC].astype(np.float16)
        # bpool one-hot, [128, NB*G]: bp[p, b*G+g] = (batch[node b*128+p]==g)
        bp = np.zeros((P, NB * G), np.float16)
        bc = batch[c * NPC:(c + 1) * NPC]
        nodes = np.arange(NPC)
        bp[nodes % P, (nodes // P) * G + bc] = 1.0
        pc = sch["per_core"][c]
        in_maps.append({
            "xT": xc.T.copy(),
            "idxA": pc["idxA"], "idxB": pc["idxB"],
            "s0": pc["s0"], "s0t": pc["s0t"],
            "eye": eye,
            "encw1": np.asarray(inputs["enc_w1"], np.float16),
            "encw2": np.asarray(inputs["enc_w2"], np.float16),
            "b1r": b1r, "gr": gr, "ber": ber, "b2r": b2r,
            "rhs0": rhs[0], "rhs1": rhs[1], "rhs2": rhs[2],
            "brep0": breps[0], "brep1": breps[1], "brep2": breps[2],
            "bpool": bp,
        })

    LAST_RESULTS["in_maps"] = in_maps
    res = run_bass_kernel_spmd(nc, in_maps, core_ids=list(range(NC)),
                               trace=TRACE)
    LAST_RESULTS["res"] = res

    pooled = np.zeros((G, D), np.float32)
    for c in range(NC):
        pooled += res.results[c]["pooled"]
    cnt = np.bincount(batch, minlength=G).astype(np.float32)[:, None]
    pooled = pooled / np.maximum(cnt, 1.0)

    # decoder MLP on host (f32, matches reference ops)
    w1 = np.asarray(inputs["dec_w1"], np.float32)
    b1 = np.asarray(inputs["dec_b1"], np.float32)
    g_ = np.asarray(inputs["dec_g"], np.float32)
    be = np.asarray(inputs["dec_be"], np.float32)
    w2 = np.asarray(inputs["dec_w2"], np.float32)
    b2 = np.asarray(inputs["dec_b2"], np.float32)
    t = pooled @ w1 + b1
    m = t.mean(-1, keepdims=True)
    v = np.square(t - m).mean(-1, keepdims=True)
    t = g_ * (t - m) / np.sqrt(v + EPS) + be
    t = np.maximum(t, 0.0)
    out = t @ w2 + b2
    return out.astype(np.float32)


# revision 8
# speedup vs baseline: 1.7458x; 1.0144x over previous
"""Trainium2 Bass kernel for nn_GATSuper (3-layer GAT + encoder/decoder MLPs).

Strategy (8 NeuronCores, SPMD):
  - Nodes sharded: core c owns global nodes [c*6250, (c+1)*6250), padded to 6272.
  - Node feature table rows are 512B: [h' 256 fp8e4m3 | al_s 4 f16 | pad].
    fp8 features cut gather traffic 33% vs f16 and let the aggregation
    matmul run at fp8 rate.
  - The gathered table is built with TWO AllGathers per layer so each can
    overlap compute: table A = each core's node positions [0,3200) (rows
    owner*3200+pos, 25600 total), table B = positions [3200,6272) (rows
    owner*3072+(pos-3200), 24576 total). Both < 32768 so dma_gather's
    int16 indices work without further splitting. AG-A fires after the
    W phase of node blocks 0..24, AG-B after blocks 25..48.
  - Edges partitioned by dst owner; within a core, grouped per dst block
    (128 dst nodes), each block's slots = [A-half tiles | B-half tiles].
  - Per layer edge phase (per dst block): dma_gather rows per edge slot,
    per-edge al_d via one-hot S0T matmul, scores s=al_s+al_d, w =
    exp(leaky_relu(s)) (f16), w cast to fp8 in-row, features weighted by w
    per head (fp8 DVE), per-block one-hot S0 aggregation matmul (fp8) also
    yields softmax denominators; divide, bias, ELU in f16; result written
    to SBUF-resident h (f16). The next layer's W matmul for the block is
    emitted inline so AllGathers and W work hide inside the edge phase.
  - Global mean pool partials per core via one-hot matmul; host sums
    partials, divides by counts, runs the decoder MLP.
"""
import sys

import ml_dtypes
import numpy as np

sys.path.insert(0, "/opt/trn_rl_repo")

from concourse import bass, bacc, mybir, tile  # noqa: E402
from concourse.bass_utils import run_bass_kernel_spmd  # noqa: E402

# ---------------- problem constants (hardcoded) ----------------
N, E, IN, HID, H, OUT, G = 50000, 800000, 128, 64, 4, 40, 8
D = HID * H  # 256
NEG_SLOPE = 0.2
EPS = 1e-5
NC = 8          # cores
P = 128
NPC = N // NC   # 6250 real nodes per core
NB = 49         # dst blocks per core (ceil(6250/128))
NPAD = NB * P   # 6272 padded nodes per core
APOS = 3200     # node positions [0,APOS) -> table A
BPOS = NPAD - APOS  # 3072 positions -> table B
AROWS = NC * APOS   # 25600
BROWS = NC * BPOS   # 24576
ROWB = 512          # bytes (= fp8 elements) per table row
WCOL = 264          # fp8 col where w (fp8) is written per edge row
RHSW = WCOL + H     # 268: agg matmul rhs width

F32 = mybir.dt.float32
F16 = mybir.dt.float16
F8 = mybir.dt.float8e4
I16 = mybir.dt.int16
AFT = mybir.ActivationFunctionType
ALU = mybir.AluOpType

TRACE = False
LAST_RESULTS = {}

_CACHE = {}


# ================= host-side schedule =================

def _build_schedule(edge_index):
    """Partition edges; build per-core gather index / one-hot arrays."""
    src = np.concatenate([edge_index[0], np.arange(N, dtype=np.int64)])
    dst = np.concatenate([edge_index[1], np.arange(N, dtype=np.int64)])

    owner = dst // NPC
    blk = (dst % NPC) // P
    dloc = (dst % NPC) % P
    s_owner = src // NPC
    s_pos = src % NPC
    half = (s_pos >= APOS).astype(np.int64)
    tabidx = np.where(half == 0, s_owner * APOS + s_pos,
                      s_owner * BPOS + (s_pos - APOS))

    # key = ((owner*NB + blk)*2 + half); count per key
    key = ((owner * NB + blk) * 2 + half)
    nkeys = NC * NB * 2
    counts = np.bincount(key, minlength=nkeys).reshape(NC, NB, 2)

    # uniform tiles per (block, half) across cores
    T = np.ceil(counts.max(axis=0) / P).astype(np.int64)  # [NB, 2]
    T = np.maximum(T, 1)

    # global chunk order: per block b: A tiles then B tiles
    chunk_ranges = {}
    a_off = {}
    b_off = {}
    acc = accA = accB = 0
    for b in range(NB):
        chunk_ranges[(b, 0)] = (acc, int(T[b, 0]))
        acc += int(T[b, 0])
        chunk_ranges[(b, 1)] = (acc, int(T[b, 1]))
        acc += int(T[b, 1])
        a_off[b] = accA
        accA += int(T[b, 0]) * P
        b_off[b] = accB
        accB += int(T[b, 1]) * P
    Ttot = acc
    slots_tot = Ttot * P

    # per-core slot arrays
    order = np.lexsort((half, blk, owner))
    src_sorted = tabidx[order]
    dl_sorted = dloc[order]
    own_sorted = owner[order]
    blk_sorted = blk[order]
    half_sorted = half[order]

    k_sorted = ((own_sorted * NB + blk_sorted) * 2 + half_sorted)
    run_starts = np.searchsorted(k_sorted, np.arange(nkeys))
    run_ends = np.searchsorted(k_sorted, np.arange(nkeys) + 1)

    per_core = []
    for c in range(NC):
        slot_src = np.zeros(slots_tot, np.int16)
        slot_dl = np.full(slots_tot, P + 1, np.int64)  # pad -> no one-hot
        slot_half = np.zeros(slots_tot, np.int8)
        for b in range(NB):
            for h in (0, 1):
                kidx = (c * NB + b) * 2 + h
                s, e = run_starts[kidx], run_ends[kidx]
                n = e - s
                c0, nt = chunk_ranges[(b, h)]
                off = c0 * P
                slot_src[off:off + n] = src_sorted[s:e]
                slot_dl[off:off + n] = dl_sorted[s:e]
                slot_half[off:off + nt * P] = h

        def wrap(a):
            return a.reshape(-1, 16).T.copy()  # [16, n/16]

        maskA = slot_half == 0
        idxA = np.tile(wrap(slot_src[maskA]), (8, 1))
        idxB = np.tile(wrap(slot_src[~maskA]), (8, 1))
        # one-hot S0 tiles: s0[t, e, d] = (dloc[slot]==d) fp8
        dl_i = slot_dl.reshape(Ttot, P)
        s0 = np.zeros((Ttot, P, P), ml_dtypes.float8_e4m3)
        tt, ee = np.nonzero(dl_i < P)
        s0[tt, ee, dl_i[tt, ee]] = 1.0
        s0_in = s0.transpose(1, 0, 2).reshape(P, Ttot * P).copy()
        s0t_in = s0.transpose(2, 0, 1).reshape(P, Ttot * P).copy()
        per_core.append(dict(idxA=idxA, idxB=idxB, s0=s0_in, s0t=s0t_in))

    return dict(T=T, chunk_ranges=chunk_ranges, Ttot=Ttot,
                a_off=a_off, b_off=b_off, slots_tot=slots_tot,
                nA=accA, nB=accB, per_core=per_core)


# ================= bass program =================

def _build_bass(sch):
    T = sch["T"]
    chunk_ranges = sch["chunk_ranges"]
    Ttot = sch["Ttot"]
    nA, nB = sch["nA"], sch["nB"]

    nc = bacc.Bacc(None, target_bir_lowering=False, num_devices=NC,
                   num_swdge_queues=4)

    # ---- inputs
    xT = nc.dram_tensor("xT", [P, NPAD], F16, kind="ExternalInput")
    idxA = nc.dram_tensor("idxA", [P, nA // 16], I16, kind="ExternalInput")
    idxB = nc.dram_tensor("idxB", [P, nB // 16], I16, kind="ExternalInput")
    s0_in = nc.dram_tensor("s0", [P, Ttot * P], F8, kind="ExternalInput")
    s0t_in = nc.dram_tensor("s0t", [P, Ttot * P], F8, kind="ExternalInput")
    eye_in = nc.dram_tensor("eye", [P, P], F16, kind="ExternalInput")
    encw1 = nc.dram_tensor("encw1", [IN, HID], F16, kind="ExternalInput")
    encw2 = nc.dram_tensor("encw2", [HID, HID], F16, kind="ExternalInput")
    b1r_in = nc.dram_tensor("b1r", [P, HID], F32, kind="ExternalInput")
    gr_in = nc.dram_tensor("gr", [P, HID], F32, kind="ExternalInput")
    ber_in = nc.dram_tensor("ber", [P, HID], F32, kind="ExternalInput")
    b2r_in = nc.dram_tensor("b2r", [P, HID], F32, kind="ExternalInput")
    rhs_in = [nc.dram_tensor(f"rhs{l}", [HID if l == 0 else D, D + 2 * H],
                             F16, kind="ExternalInput") for l in range(3)]
    brep_in = [nc.dram_tensor(f"brep{l}", [P, D], F16, kind="ExternalInput")
               for l in range(3)]
    bpool_in = nc.dram_tensor("bpool", [P, NB * G], F16, kind="ExternalInput")

    pooled_out = nc.dram_tensor("pooled", [G, D], F32, kind="ExternalOutput")

    with tile.TileContext(nc) as tc:
        with tc.tile_pool(name="const", bufs=1) as cst, \
             tc.tile_pool(name="hwork", bufs=3) as hwork, \
             tc.tile_pool(name="lhsT", bufs=3) as lhsp, \
             tc.tile_pool(name="hcat", bufs=3) as hcatp, \
             tc.tile_pool(name="gbuf", bufs=6) as gbuf, \
             tc.tile_pool(name="s0b", bufs=6) as s0buf, \
             tc.tile_pool(name="small", bufs=4) as smallp, \
             tc.tile_pool(name="outp", bufs=3) as outp, \
             tc.tile_pool(name="pt", bufs=1, space="PSUM") as pt, \
             tc.tile_pool(name="pw", bufs=2, space="PSUM") as pw, \
             tc.tile_pool(name="pe", bufs=3, space="PSUM") as pep, \
             tc.tile_pool(name="pad", bufs=1, space="PSUM") as pad, \
             tc.tile_pool(name="pp", bufs=1, space="PSUM") as ppool, \
             tc.tile_pool(name="dram", bufs=1, space="DRAM") as dram:

            # ---- load constants
            def load(t_in, shape, nm, dt=F32):
                t = cst.tile(shape, dt, name=nm)
                nc.sync.dma_start(t[:], t_in[:])
                return t

            idxA_t = load(idxA, [P, nA // 16], "idxA_t", I16)
            idxB_t = load(idxB, [P, nB // 16], "idxB_t", I16)
            eye_t = load(eye_in, [P, P], "eye_t", F16)
            encw1_t = load(encw1, [IN, HID], "encw1_t", F16)
            encw2_t = load(encw2, [HID, HID], "encw2_t", F16)
            b1r_t = load(b1r_in, [P, HID], "b1r_t")
            gr_t = load(gr_in, [P, HID], "gr_t")
            ber_t = load(ber_in, [P, HID], "ber_t")
            b2r_t = load(b2r_in, [P, HID], "b2r_t")
            bpool_t = load(bpool_in, [P, NB * G], "bpool_t", F16)
            rhs_t = []
            for l in range(3):
                if l == 0:
                    r0 = cst.tile([HID, D + 2 * H], F16, name=f"rhsL{l}")
                    nc.sync.dma_start(r0[:], rhs_in[l][:])
                    rhs_t.append([r0])
                else:
                    chunks = []
                    for cch in range(D // P):
                        rc = cst.tile([P, D + 2 * H], F16,
                                      name=f"rhsL{l}c{cch}")
                        nc.sync.dma_start(
                            rc[:], rhs_in[l][cch * P:(cch + 1) * P, :])
                        chunks.append(rc)
                    rhs_t.append(chunks)
            brep_t = [load(brep_in[l], [P, D], f"brep_t{l}", F16)
                      for l in range(3)]
            h0_t = cst.tile([P, NB * HID], F16)  # encoder out, SBUF-resident
            h_sb = cst.tile([P, NB * D], F16)    # GAT layer io, SBUF-resident
            ald_t = cst.tile([P, NB * H], F16)   # per-layer al_d

            # ---- DRAM scratch
            hcat_own_l = [dram.tile([NPAD, ROWB], F8, name=f"hcown{l}")
                          for l in range(3)]
            hcatA_l = [dram.tile([AROWS, ROWB], F8, name=f"hcA{l}",
                                 addr_space="Shared") for l in range(3)]
            hcatB_l = [dram.tile([BROWS, ROWB], F8, name=f"hcB{l}",
                                 addr_space="Shared") for l in range(3)]

            def emit_ag(l, half):
                own = hcat_own_l[l]
                if half == 0:
                    nc.gpsimd.collective_compute(
                        "AllGather", ALU.bypass,
                        replica_groups=[list(range(NC))],
                        ins=[own[0:APOS, :].opt()],
                        outs=[hcatA_l[l][:].opt()])
                else:
                    nc.gpsimd.collective_compute(
                        "AllGather", ALU.bypass,
                        replica_groups=[list(range(NC))],
                        ins=[own[APOS:NPAD, :].opt()],
                        outs=[hcatB_l[l][:].opt()])

            def emit_w_block(l, n, rhs_chunks):
                """W matmul for layer l, node block n -> hcat_own[l] rows."""
                psw = pw.tile([P, D + 2 * H], F32, space="PSUM", tag="pw")
                if l == 0:
                    tin = h0_t[:, n * HID:(n + 1) * HID]
                    pst = pt.tile([HID, P], F16, space="PSUM", tag="pt")
                    lt = lhsp.tile([HID, P], F16, tag="lt64")
                    nc.tensor.transpose(pst[:], tin, eye_t[:])
                    nc.scalar.activation(lt[:], pst[:], AFT.Copy)
                    nc.tensor.matmul(psw[:], lhsT=lt[:], rhs=rhs_chunks[0][:],
                                     start=True, stop=True)
                else:
                    for cch in range(2):
                        tin = h_sb[:, n * D + cch * P:n * D + (cch + 1) * P]
                        pst = pt.tile([P, P], F16, space="PSUM", tag="pt")
                        lt = lhsp.tile([P, P], F16, tag="lt128")
                        nc.tensor.transpose(pst[:], tin, eye_t[:])
                        nc.scalar.activation(lt[:], pst[:], AFT.Copy)
                        nc.tensor.matmul(psw[:], lhsT=lt[:],
                                         rhs=rhs_chunks[cch][:],
                                         start=(cch == 0), stop=(cch == 1))
                hct = hcatp.tile([P, ROWB], F8, tag="hc")
                nc.scalar.activation(hct[:, 0:D], psw[:, 0:D], AFT.Copy)
                hct16 = hct[:].bitcast(F16)
                nc.scalar.activation(hct16[:, D // 2:D // 2 + H],
                                     psw[:, D:D + H], AFT.Copy)
                nc.scalar.activation(ald_t[:, n * H:(n + 1) * H],
                                     psw[:, D + H:D + 2 * H], AFT.Copy)
                nc.sync.dma_start(hcat_own_l[l][n * P:(n + 1) * P, :], hct[:])

            # ================ encoder + layer-0 W phase ================
            for n in range(NB):
                xt_blk = hwork.tile([P, P], F16, tag="xt")
                nc.sync.dma_start(xt_blk[:], xT[:, n * P:(n + 1) * P])
                psum1 = pw.tile([P, HID], F32, space="PSUM", tag="pw")
                nc.tensor.matmul(psum1[:], lhsT=xt_blk[:],
                                 rhs=encw1_t[:], start=True, stop=True)
                t = hwork.tile([P, HID], F32, tag="enc")
                nc.vector.tensor_tensor(out=t[:], in0=psum1[:], in1=b1r_t[:],
                                        op=ALU.add)
                # layernorm over HID
                mean = smallp.tile([P, 1], F32, tag="m")
                nc.vector.reduce_sum(out=mean[:], in_=t[:],
                                     axis=mybir.AxisListType.X)
                nc.vector.tensor_scalar_mul(mean[:], mean[:], 1.0 / HID)
                nc.vector.tensor_scalar(out=t[:], in0=t[:], scalar1=mean[:],
                                        scalar2=None, op0=ALU.subtract)
                sq = hwork.tile([P, HID], F32, tag="sq")
                nc.scalar.square(sq[:], t[:])
                var = smallp.tile([P, 1], F32, tag="v")
                nc.vector.reduce_sum(out=var[:], in_=sq[:],
                                     axis=mybir.AxisListType.X)
                nc.vector.tensor_scalar(out=var[:], in0=var[:],
                                        scalar1=1.0 / HID, scalar2=EPS,
                                        op0=ALU.mult, op1=ALU.add)
                nc.scalar.sqrt(var[:], var[:])
                nc.vector.reciprocal(var[:], var[:])
                nc.vector.tensor_scalar(out=t[:], in0=t[:], scalar1=var[:],
                                        scalar2=None, op0=ALU.mult)
                nc.vector.tensor_tensor(out=t[:], in0=t[:], in1=gr_t[:],
                                        op=ALU.mult)
                nc.vector.tensor_tensor(out=t[:], in0=t[:], in1=ber_t[:],
                                        op=ALU.add)
                t16 = hwork.tile([P, HID], F16, tag="enc16")
                nc.scalar.activation(t16[:], t[:], AFT.Relu)
                pst = pt.tile([HID, P], F16, space="PSUM", tag="pt")
                nc.tensor.transpose(pst[:], t16[:], eye_t[:])
                lt = lhsp.tile([HID, P], F16, tag="lt64")
                nc.scalar.activation(lt[:], pst[:], AFT.Copy)
                psum2 = pw.tile([P, HID], F32, space="PSUM", tag="pw")
                nc.tensor.matmul(psum2[:], lhsT=lt[:], rhs=encw2_t[:],
                                 start=True, stop=True)
                nc.vector.tensor_tensor(out=h0_t[:, n * HID:(n + 1) * HID],
                                        in0=psum2[:], in1=b2r_t[:], op=ALU.add)
                emit_w_block(0, n, rhs_t[0])
                if n == 24:
                    emit_ag(0, 0)
            emit_ag(0, 1)

            # ================ GAT layers ================
            qn = [0]

            def next_q():
                qn[0] = (qn[0] + 1) % 4
                return qn[0]

            # Two-stage software pipeline per layer: stage 1 (gather, scores,
            # weighting, aggregation matmuls) for block b+1 is emitted BEFORE
            # stage 2 (softmax divide, ELU, next-layer W) of block b, so each
            # in-order engine queue interleaves two blocks' work instead of
            # serializing on the full per-block dependency chain.
            def edge_stage0(l, b):
                cA0, tA = chunk_ranges[(b, 0)]
                cB0, tB = chunk_ranges[(b, 1)]
                cap = tA + tB
                g0 = cA0
                gt = gbuf.tile([P, cap * ROWB], F8, tag="g")
                g3 = gt[:].rearrange("p (r c) -> p r c", r=cap)
                offA = sch["a_off"][b]
                offB = sch["b_off"][b]
                # split each half-gather in two so all 4 SWDGE queues
                # (= 4 Q7 core pairs) generate descriptors in parallel
                parts = []
                tA1 = tA // 2
                if tA1:
                    parts.append((hcatA_l[l], idxA_t, offA, 0, tA1))
                parts.append((hcatA_l[l], idxA_t, offA + tA1 * P, tA1,
                              tA - tA1))
                tB1 = tB // 2
                if tB1:
                    parts.append((hcatB_l[l], idxB_t, offB, tA, tB1))
                parts.append((hcatB_l[l], idxB_t, offB + tB1 * P,
                              tA + tB1, tB - tB1))
                for src_t, idx_t, off, t0, nt in parts:
                    nc.gpsimd.dma_gather(
                        out_ap=g3[:, t0:t0 + nt, :],
                        in_ap=src_t[:],
                        idxs_ap=idx_t[:, off // 16:(off + nt * P) // 16],
                        num_idxs=nt * P, num_idxs_reg=nt * P,
                        elem_size=ROWB, single_packet=False,
                        queue_num=next_q())
                # one-hot tiles for this block
                s0g = s0buf.tile([P, cap * P], F8, tag="s0g")
                nc.scalar.dma_start(s0g[:], s0_in[:, g0 * P:(g0 + cap) * P])
                s0tg = s0buf.tile([P, cap * P], F8, tag="s0tg")
                nc.sync.dma_start(s0tg[:], s0t_in[:, g0 * P:(g0 + cap) * P])
                return dict(gt=gt, g3=g3, s0g=s0g, s0tg=s0tg, cap=cap)

            def edge_stage1(l, b, ctx):
                gt, g3 = ctx["gt"], ctx["g3"]
                s0g, s0tg, cap = ctx["s0g"], ctx["s0tg"], ctx["cap"]
                # per-edge al_d via PE
                adp = pad.tile([P, cap * H], F32, space="PSUM", tag="adp")
                for t_ in range(cap):
                    nc.tensor.matmul(
                        adp[:, t_ * H:(t_ + 1) * H],
                        lhsT=s0tg[:, t_ * P:(t_ + 1) * P],
                        rhs=ald_t[:, b * H:(b + 1) * H],
                        start=True, stop=True)
                # scores: copy strided al_s to a compact buffer (scalar
                # engine handles the strided access), then contiguous DVE
                g16 = gt[:].bitcast(F16).rearrange("p (r c) -> p r c", r=cap)
                sl = g16[:, :, D // 2:D // 2 + H]
                sc = smallp.tile([P, cap * H], F16, tag="sc")
                sc3 = sc[:].rearrange("p (r c) -> p r c", r=cap)
                nc.scalar.activation(sc3, sl, AFT.Copy)
                nc.vector.tensor_tensor(out=sc[:], in0=sc[:], in1=adp[:],
                                        op=ALU.add)
                # leaky relu: max(s, 0.2*s)
                tmp = smallp.tile([P, cap * H], F16, tag="lrl")
                nc.vector.tensor_scalar_mul(tmp[:], sc[:], NEG_SLOPE)
                nc.vector.tensor_tensor(out=sc[:], in0=sc[:], in1=tmp[:],
                                        op=ALU.max)
                # w = exp(s), written straight into row col WCOL as fp8
                w8 = g3[:, :, WCOL:WCOL + H]
                nc.scalar.activation(w8, sc3, AFT.Exp)
                # weight features by w per head (fp8 in-place)
                for hh in range(H):
                    in0 = g3[:, :, hh * HID:(hh + 1) * HID]
                    in1 = g3[:, :, WCOL + hh:WCOL + hh + 1].to_broadcast(
                        [P, cap, HID])
                    nc.vector.tensor_tensor(out=in0, in0=in0, in1=in1,
                                            op=ALU.mult)
                # per-block aggregation matmul (fp8)
                pse = pep.tile([P, RHSW], F32, space="PSUM", tag="pe")
                for i in range(cap):
                    nc.tensor.matmul(pse[:],
                                     lhsT=s0g[:, i * P:(i + 1) * P],
                                     rhs=g3[:, i, 0:RHSW],
                                     start=(i == 0), stop=(i == cap - 1))
                return pse

            def edge_stage2(l, b, pse, psp):
                # softmax divide + bias + ELU (f16)
                den = smallp.tile([P, H], F32, tag="den")
                nc.vector.tensor_scalar(out=den[:],
                                        in0=pse[:, WCOL:WCOL + H],
                                        scalar1=1e-16, scalar2=None,
                                        op0=ALU.add)
                nc.vector.reciprocal(den[:], den[:])
                xo = outp.tile([P, D], F16, tag="xo")
                den_b = den[:].rearrange(
                    "p (h o) -> p h o", o=1).to_broadcast([P, H, HID])
                nc.vector.tensor_tensor(
                    out=xo[:].rearrange("p (h c) -> p h c", h=H),
                    in0=pse[:, 0:D].rearrange("p (h c) -> p h c", h=H),
                    in1=den_b, op=ALU.mult)
                nc.vector.tensor_tensor(out=xo[:], in0=xo[:],
                                        in1=brep_t[l][:], op=ALU.add)
                # ELU: (relu(x)-1) + exp(x-relu(x))
                r = outp.tile([P, D], F16, tag="r")
                nc.scalar.activation(r[:], xo[:], AFT.Relu)
                m = outp.tile([P, D], F16, tag="m")
                nc.vector.tensor_tensor(out=m[:], in0=xo[:], in1=r[:],
                                        op=ALU.subtract)
                nc.scalar.activation(m[:], m[:], AFT.Exp)
                nc.vector.tensor_scalar(out=r[:], in0=r[:], scalar1=-1.0,
                                        scalar2=None, op0=ALU.add)
                hout = h_sb[:, b * D:(b + 1) * D]
                nc.vector.tensor_tensor(out=hout, in0=r[:],
                                        in1=m[:], op=ALU.add)
                if l < 2:
                    # next layer W phase for this block, inline
                    emit_w_block(l + 1, b, rhs_t[l + 1])
                    if b == 24:
                        emit_ag(l + 1, 0)
                    elif b == NB - 1:
                        emit_ag(l + 1, 1)
                else:
                    # pooling partial: psp += Bp.T @ h
                    nc.tensor.matmul(
                        psp[:], lhsT=bpool_t[:, b * G:(b + 1) * G],
                        rhs=hout, start=(b == 0), stop=(b == NB - 1))
                    if b == NB - 1:
                        po = outp.tile([G, D], F32, tag="po")
                        nc.scalar.activation(po[:], psp[:], AFT.Copy)
                        nc.sync.dma_start(pooled_out[:], po[:])

            for l in range(3):
                psp = None
                if l == 2:
                    psp = ppool.tile([G, D], F32, space="PSUM", name="psp")
                ctx0 = edge_stage0(l, 0)
                ctx1 = edge_stage0(l, 1)
                pse0 = edge_stage1(l, 0, ctx0)
                for b in range(2, NB):
                    ctx2 = edge_stage0(l, b)
                    pse1 = edge_stage1(l, b - 1, ctx1)
                    edge_stage2(l, b - 2, pse0, psp)
                    ctx1, pse0 = ctx2, pse1
                pse1 = edge_stage1(l, NB - 1, ctx1)
                edge_stage2(l, NB - 2, pse0, psp)
                edge_stage2(l, NB - 1, pse1, psp)
    return nc


# ================= host wrapper =================

def kernel(**inputs):
    x = np.asarray(inputs["x"], np.float32)
    edge_index = np.asarray(inputs["edge_index"]).astype(np.int64)
    batch = np.asarray(inputs["batch"]).astype(np.int64)

    if "sch" not in _CACHE:
        _CACHE["sch"] = _build_schedule(edge_index)
        _CACHE["nc"] = _build_bass(_CACHE["sch"])
        _CACHE["nc"].compile()
    sch = _CACHE["sch"]
    nc = _CACHE["nc"]

    # ---- weight prep
    def a_tilde(a):  # [H, HID] -> [D, H] block diag
        m = np.zeros((D, H), np.float32)
        for h in range(H):
            m[h * HID:(h + 1) * HID, h] = a[h]
        return m

    rhs = []
    breps = []
    for l in range(3):
        W = np.asarray(inputs[f"conv{l}_w"], np.float32)
        a_s = np.asarray(inputs[f"conv{l}_as"], np.float32)
        a_d = np.asarray(inputs[f"conv{l}_ad"], np.float32)
        bb = np.asarray(inputs[f"conv{l}_b"], np.float32)
        rhs.append(np.concatenate(
            [W, W @ a_tilde(a_s), W @ a_tilde(a_d)], axis=1).astype(np.float16))
        breps.append(np.tile(bb[None, :], (P, 1)).astype(np.float16))

    eye = np.eye(P, dtype=np.float16)
    b1r = np.tile(np.asarray(inputs["enc_b1"], np.float32)[None, :], (P, 1))
    gr = np.tile(np.asarray(inputs["enc_g"], np.float32)[None, :], (P, 1))
    ber = np.tile(np.asarray(inputs["enc_be"], np.float32)[None, :], (P, 1))
    b2r = np.tile(np.asarray(inputs["enc_b2"], np.float32)[None, :], (P, 1))

    in_maps = []
    for c in range(NC):
        xc = np.zeros((NPAD, IN), np.float16)
        xc[:NPC] = x[c * NPC:(c + 1) * NPC].astype(np.float16)
        # bpool one-hot, [128, NB*G]: bp[p, b*G+g] = (batch[node b*128+p]==g)
        bp = np.zeros((P, NB * G), np.float16)
        bc = batch[c * NPC:(c + 1) * NPC]
        nodes = np.arange(NPC)
        bp[nodes % P, (nodes // P) * G + bc] = 1.0
        pc = sch["per_core"][c]
        in_maps.append({
            "xT": xc.T.copy(),
            "idxA": pc["idxA"], "idxB": pc["idxB"],
            "s0": pc["s0"], "s0t": pc["s0t"],
            "eye": eye,
            "encw1": np.asarray(inputs["enc_w1"], np.float16),
            "encw2": np.asarray(inputs["enc_w2"], np.float16),
            "b1r": b1r, "gr": gr, "ber": ber, "b2r": b2r,
            "rhs0": rhs[0], "rhs1": rhs[1], "rhs2": rhs[2],
            "brep0": breps[0], "brep1": breps[1], "brep2": breps[2],
            "bpool": bp,
        })

    LAST_RESULTS["in_maps"] = in_maps
    res = run_bass_kernel_spmd(nc, in_maps, core_ids=list(range(NC)),
                               trace=TRACE)
    LAST_RESULTS["res"] = res

    pooled = np.zeros((G, D), np.float32)
    for c in range(NC):
        pooled += res.results[c]["pooled"]
    cnt = np.bincount(batch, minlength=G).astype(np.float32)[:, None]
    pooled = pooled / np.maximum(cnt, 1.0)

    # decoder MLP on host (f32, matches reference ops)
    w1 = np.asarray(inputs["dec_w1"], np.float32)
    b1 = np.asarray(inputs["dec_b1"], np.float32)
    g_ = np.asarray(inputs["dec_g"], np.float32)
    be = np.asarray(inputs["dec_be"], np.float32)
    w2 = np.asarray(inputs["dec_w2"], np.float32)
    b2 = np.asarray(inputs["dec_b2"], np.float32)
    t = pooled @ w1 + b1
    m = t.mean(-1, keepdims=True)
    v = np.square(t - m).mean(-1, keepdims=True)
    t = g_ * (t - m) / np.sqrt(v + EPS) + be
    t = np.maximum(t, 0.0)
    out = t @ w2 + b2
    return out.astype(np.float32)
